# revision 19
# baseline (speedup 1.0000x reference)
"""DiT block kernel for 8 Trainium2 NeuronCores (Bass/Tile).

Sharding: sequence dim L=4096 split 8 ways (512 query rows per core).
Each core computes K/V for the full sequence (replicated compute; on
this stack a single cross-core collective costs ~50us of latency, more
than the ~60us of fully-pipelined extra matmul work it would save, so
no collectives are used). Inputs are rotated host-side so every core's
local rows sit at positions [0, 512) -> one SPMD program.

Structure: the LN1 -> xn1T8 -> V projection runs as a stream over 8
sequence blocks (VectorE stats, Pool normalize, DMA transposes, 3-way
modulate split); Q and K0 follow; K for pair hp+1 is emitted inside
attention pair hp where the PE has slack (attention is exp-bound).
Softmax exp alternates ScalarE (native Exp) / VectorE (Schraudolph
int8-bitcast) -- the only two engines that can read PSUM. AdaLN runs
in transposed orientation (few wide matmuls instead of many tiny
ones), shift/scale-1 up front, the rest hidden inside attention.

Precision: fp8e4 (e4m3) DoubleRow matmuls for QKV/out-proj/FFN; bf16
scores with the two heads of a pair on disjoint PE row-groups
(concurrent); fp8 AV (DoubleRow over key-chunk pairs); fp32 PSUM
accumulation; fp32 layernorm stats and residuals; attention output
scaled by 16 before the fp8 cast (undone in the out-projection).
"""

import sys

sys.path.insert(0, "/opt/trn_rl_repo")

from contextlib import ExitStack

import numpy as np
import ml_dtypes

import concourse.bass as bass
import concourse.bacc as bacc
import concourse.tile as tile
import concourse.mybir as mybir
from concourse.bass_utils import run_bass_kernel_spmd

F32 = mybir.dt.float32
BF16 = mybir.dt.bfloat16
FP8 = mybir.dt.float8e4
FP8E5 = mybir.dt.float8e5
I8 = mybir.dt.int8
AF = mybir.ActivationFunctionType
OP = mybir.AluOpType
DR = mybir.MatmulPerfMode.DoubleRow

L, D, H, HD, DM = 4096, 768, 12, 64, 3072
NCORES = 8
LQ = L // NCORES  # 512 local query rows
P = 128
EPS = 1e-5
NKC = L // P  # 32 k-chunks of 128
NQC = LQ // P  # 4 local q-chunks of 128
NDC = D // P  # 6 chunks of the model dim
NHP = H // 2  # 6 head pairs
NMC = DM // P  # 24 chunks of the FFN hidden dim
NBL = L // LQ  # 8 sequence blocks of 512
VE = HD + 4  # V row stride (64 dims + ones + pad)

SW_QKV = 64.0
SW_AO = 16.0
SW_F1 = 16.0
SW_F2 = 32.0
SW_CAT = 16.0  # ones-row = 1/SW_CAT so cat lands in fp8e4's sweet spot

# Schraudolph exp -> fp8e5 bits: i8 = round(x*2^2/ln2*0.125 + C)
SCH_A = 5.770780 * 0.125
SCH_B = 59.77


def _declare_params(nc):
    dp = nc.declare_dram_parameter
    t = {}
    t["x"] = dp("x", [LQ, D], F32, isOutput=False)
    t["x_bf"] = dp("x_bf", [L, D], BF16, isOutput=False)
    t["cond_t"] = dp("cond_t", [P, NDC], F32, isOutput=False)
    t["wad1"] = dp("wad1", [P, NDC, 2 * D], BF16, isOutput=False)  # sh1|sc1 cols
    t["wad2"] = dp("wad2", [P, NDC, 4 * D], BF16, isOutput=False)  # sh2|sc2|g1|g2
    t["bad1"] = dp("bad1", [1, 2 * D], F32, isOutput=False)
    t["bad2sh"] = dp("bad2sh", [P, 2 * NDC], F32, isOutput=False)
    t["bad2g"] = dp("bad2g", [1, 4 * 384], BF16, isOutput=False)
    t["w_qkv8"] = dp("w_qkv8", [D, 3 * D], FP8, isOutput=False)  # * SW_QKV
    t["b_q_col"] = dp("b_q_col", [P, NDC], F32, isOutput=False)
    t["w_ao8"] = dp("w_ao8", [D, D], FP8, isOutput=False)  # * SW_AO
    t["b_attn_b"] = dp("b_attn_b", [P, D], F32, isOutput=False)  # b_attn + bv@Wao
    t["w_ffn18"] = dp("w_ffn18", [D, DM], FP8, isOutput=False)  # * SW_F1
    t["b_ffn1_col"] = dp("b_ffn1_col", [P, NMC], F32, isOutput=False)
    t["w_f28"] = dp("w_f28", [DM, D], FP8, isOutput=False)  # * SW_F2
    t["b_ffn2_b"] = dp("b_ffn2_b", [P, D], F32, isOutput=False)
    t["out"] = dp("out", [LQ, D], F32, isOutput=True)
    return t


def _build_body(nc, tc, ctx, t):
    mm = nc.tensor.matmul
    dma = nc.sync.dma_start
    dma2 = nc.scalar.dma_start  # second HWDGE queue
    dmat = nc.sync.dma_start_transpose
    v = nc.vector
    gp = nc.gpsimd
    act = nc.scalar.activation

    const = ctx.enter_context(tc.tile_pool(name="const", bufs=1))
    eps_t = const.tile([P, 1], F32)
    v.memset(eps_t, EPS)

    dram = ctx.enter_context(tc.tile_pool(name="dram", bufs=1, space="DRAM"))
    drA = dram.tile([2 * D], F32)  # adaln1 row bounce
    drB = dram.tile([2 * D], F32)  # adaln2 sh/sc row bounce

    adaln = ctx.enter_context(tc.tile_pool(name="adaln", bufs=1))
    sh1_col = adaln.tile([P, NDC], F32)
    sp1_col = adaln.tile([P, NDC], F32)
    sh2_col = adaln.tile([P, NDC], F32)
    sp2_col = adaln.tile([P, NDC], F32)
    g1s_b = adaln.tile([P, D], F32)
    g2s_b = adaln.tile([P, D], F32)
    xb_bias = adaln.tile([P, D], F32)
    x2b_bias = adaln.tile([P, D], F32)
    sc_bf = adaln.tile([P, NDC], BF16)

    # ---------------- phase A: adaln1 (sh1/sc1 only) ----------------------
    with ExitStack() as phA:
        pool = phA.enter_context(tc.tile_pool(name="phA", bufs=1))
        psA = phA.enter_context(tc.tile_pool(name="psA", bufs=2, space="PSUM"))

        cond_sb = pool.tile([P, NDC], F32)
        dma(out=cond_sb[:], in_=t["cond_t"][:])
        sc_f = pool.tile([P, NDC], F32)
        act(sc_f[:], cond_sb[:], AF.Silu)
        v.tensor_copy(sc_bf[:], sc_f[:])

        wad1 = pool.tile([P, NDC, 2 * D], BF16)
        dma2(out=wad1[:], in_=t["wad1"][:])
        b1row = pool.tile([1, 2 * D], F32)
        dma(out=b1row[:], in_=t["bad1"][:])

        a1row = pool.tile([1, 2 * D], F32)
        for tt in range(3):
            ps = psA.tile([1, 512], F32)
            for dc in range(NDC):
                mm(
                    ps[:],
                    sc_bf[:, dc : dc + 1],
                    wad1[:, dc, tt * 512 : (tt + 1) * 512],
                    start=(dc == 0),
                    stop=(dc == NDC - 1),
                )
            v.tensor_add(
                a1row[:, tt * 512 : (tt + 1) * 512], ps[:],
                b1row[:, tt * 512 : (tt + 1) * 512],
            )
        dma(out=drA[:].rearrange("(o d) -> o d", o=1), in_=a1row[:])
        dma(out=sh1_col[:], in_=drA[0:D].rearrange("(c p) -> p c", p=P))
        sp1_raw = pool.tile([P, NDC], F32)
        dma(out=sp1_raw[:], in_=drA[D : 2 * D].rearrange("(c p) -> p c", p=P))
        v.tensor_scalar_add(sp1_col[:], sp1_raw[:], 1.0)

    # ---------------- phase B: streamed LN1 + V/Q -------------------------
    big = ctx.enter_context(tc.tile_pool(name="big", bufs=1))
    x_loc = big.tile([P, NQC, D], F32)
    x2_loc = [big.tile([P, D], F32, name=f"x2_loc{q}") for q in range(NQC)]
    catT8 = big.tile([P, NDC, LQ], FP8)
    xn2T8 = big.tile([P, NDC, LQ], FP8)

    s_attn = ctx.enter_context(ExitStack())
    attn_pool = s_attn.enter_context(tc.tile_pool(name="attn", bufs=1))
    kT_all = attn_pool.tile([P, NHP, L], BF16)
    v4 = attn_pool.tile([P, NKC, H * VE], FP8)
    qT_all = attn_pool.tile([P, NHP, LQ], BF16)
    xn1T8 = [
        attn_pool.tile([P, NDC, LQ], FP8, name=f"xn1T8_{b}") for b in range(NBL)
    ]
    w8qkv = attn_pool.tile([P, NDC, 3 * D], FP8)
    bq_col = attn_pool.tile([P, NDC], F32)

    dma(out=x_loc[:], in_=t["x"].rearrange("(n p) d -> p n d", p=P))
    dma2(out=w8qkv[:], in_=t["w_qkv8"].rearrange("(c p) m -> p c m", p=P))
    dma(out=bq_col[:], in_=t["b_q_col"][:])
    v4r = v4.rearrange("p k (h e) -> p k h e", e=VE)
    v.memset(v4r[:, :, :, HD : HD + 1], 1.0 / SW_CAT)
    v.memset(v4r[:, :, :, HD + 1 :], 0.0)

    phB = ctx.enter_context(ExitStack())
    xload = phB.enter_context(tc.tile_pool(name="xload", bufs=10))
    spool = phB.enter_context(tc.tile_pool(name="spool", bufs=10))
    nxpool = phB.enter_context(tc.tile_pool(name="nxpool", bufs=8))
    tpool = phB.enter_context(tc.tile_pool(name="tpool", bufs=2))
    psB1 = phB.enter_context(tc.tile_pool(name="psB1", bufs=2, space="PSUM"))
    psB2 = phB.enter_context(tc.tile_pool(name="psB2", bufs=2, space="PSUM"))

    x_r = t["x_bf"].rearrange("(n p) d -> n p d", p=P)
    for b in range(NBL):
        xn1T_bf = tpool.tile([P, NDC, LQ], BF16, tag="xnbf", name=f"xnbf{b}")
        nxs = []
        for ii in range(4):
            i = b * 4 + ii
            xt = xload.tile([P, D], BF16, tag="xt", name=f"xt{ii}")
            dma(out=xt[:], in_=x_r[i])
            stats = spool.tile([P, 2, 6], F32, tag="stats", name=f"st{ii}")
            for g in range(2):
                v.bn_stats(stats[:, g, :], xt[:, g * 384 : (g + 1) * 384])
            mv = spool.tile([P, 2], F32, tag="mv", name=f"mv{ii}")
            v.bn_aggr(mv[:], stats[:])
            sq = spool.tile([P, 1], F32, tag="sq", name=f"sq{ii}")
            act(sq[:], mv[:, 1:2], AF.Sqrt, bias=eps_t[:, 0:1])
            rstd = spool.tile([P, 1], F32, tag="rstd", name=f"rstd{ii}")
            v.reciprocal_approx_fast(rstd[:], sq[:])
            # normalize on Pool (SBUF->SBUF) to keep DVE free
            nx = nxpool.tile([P, D], BF16, tag="nx", name=f"nx{ii}")
            gp.tensor_scalar(
                nx[:], xt[:], mv[:, 0:1], rstd[:, 0:1],
                op0=OP.subtract, op1=OP.mult,
            )
            nxs.append(nx)
        for ii in range(4):
            dmat(out=xn1T_bf[:, :, ii * P : (ii + 1) * P], in_=nxs[ii][:])
        for dc in range(NDC):
            e3 = dc % 3
            if e3 == 0:
                act(
                    xn1T8[b][:, dc, :], xn1T_bf[:, dc, :], AF.Identity,
                    bias=sh1_col[:, dc : dc + 1], scale=sp1_col[:, dc : dc + 1],
                )
            elif e3 == 1:
                v.tensor_scalar(
                    xn1T8[b][:, dc, :], xn1T_bf[:, dc, :],
                    sp1_col[:, dc : dc + 1], sh1_col[:, dc : dc + 1],
                    op0=OP.mult, op1=OP.add,
                )
            else:
                gp.tensor_scalar(
                    xn1T8[b][:, dc, :], xn1T_bf[:, dc, :],
                    sp1_col[:, dc : dc + 1], sh1_col[:, dc : dc + 1],
                    op0=OP.mult, op1=OP.add,
                )
        # V projection for this block (4 k-chunks)
        for lc in range(4):
            ps_v = psB2.tile([P, D], F32, tag="psv", name=f"psv{lc}")
            for dc2 in range(NDC // 2):
                lhs = xn1T8[b][:, 2 * dc2 : 2 * dc2 + 2, lc * P : (lc + 1) * P]
                mm(ps_v[:, 0:512], lhs,
                   w8qkv[:, 2 * dc2 : 2 * dc2 + 2, 2 * D : 2 * D + 512],
                   start=(dc2 == 0), stop=(dc2 == NDC // 2 - 1), perf_mode=DR)
                mm(ps_v[:, 512:D], lhs,
                   w8qkv[:, 2 * dc2 : 2 * dc2 + 2, 2 * D + 512 : 3 * D],
                   start=(dc2 == 0), stop=(dc2 == NDC // 2 - 1), perf_mode=DR)
            kc = b * 4 + lc
            if lc % 2 == 0:
                act(v4r[:, kc, :, 0:HD],
                    ps_v.rearrange("p (h e) -> p h e", e=HD),
                    AF.Copy, scale=1.0 / SW_QKV)
            else:
                v.tensor_scalar(
                    v4r[:, kc, :, 0:HD],
                    ps_v.rearrange("p (h e) -> p h e", e=HD),
                    1.0 / SW_QKV, 0.0, op0=OP.mult, op1=OP.add,
                )
        if b == 0:
            # Q projection (local rows = block 0 thanks to the roll)
            for hp in range(NHP):
                ps_q = psB1.tile([P, LQ], F32, tag="psq", name=f"psq{hp}")
                for dc2 in range(NDC // 2):
                    mm(
                        ps_q[:],
                        w8qkv[:, 2 * dc2 : 2 * dc2 + 2, hp * P : (hp + 1) * P],
                        xn1T8[0][:, 2 * dc2 : 2 * dc2 + 2, :],
                        start=(dc2 == 0),
                        stop=(dc2 == NDC // 2 - 1),
                        perf_mode=DR,
                    )
                v.tensor_scalar(
                    qT_all[:, hp, :], ps_q[:], 1.0 / SW_QKV,
                    bq_col[:, hp : hp + 1], op0=OP.mult, op1=OP.add,
                )

    phB.close()  # release LN1 streaming pools before attention
    # ---------------- phase C: attention (+K emit, +adaln2) ---------------
    with ExitStack() as phC:
        pt_pool = phC.enter_context(tc.tile_pool(name="ptp", bufs=6))
        tiny = phC.enter_context(tc.tile_pool(name="tiny", bufs=2))
        psS = phC.enter_context(tc.tile_pool(name="psS", bufs=2, space="PSUM"))
        psK = phC.enter_context(tc.tile_pool(name="psK", bufs=2, space="PSUM"))
        psO = phC.enter_context(tc.tile_pool(name="psO", bufs=1, space="PSUM"))

        def emit_K(hp):
            # K for head pair hp over the full sequence; bias dropped
            # (constant per query -> softmax invariant), 1/SW at convert.
            for b in range(NBL):
                ps_k = psK.tile([P, LQ], F32, tag="ps_k", name=f"psk{b}")
                for dc2 in range(NDC // 2):
                    mm(
                        ps_k[:],
                        w8qkv[:, 2 * dc2 : 2 * dc2 + 2, D + hp * P : D + (hp + 1) * P],
                        xn1T8[b][:, 2 * dc2 : 2 * dc2 + 2, :],
                        start=(dc2 == 0),
                        stop=(dc2 == NDC // 2 - 1),
                        perf_mode=DR,
                    )
                act(kT_all[:, hp, b * LQ : (b + 1) * LQ], ps_k[:],
                    AF.Copy, scale=1.0 / SW_QKV)

        def emit_adaln2():
            # 8 tiles of 384 cols aligned to [sh2|sc2|g1|g2] groups
            with tc.tile_pool(name="wadp", bufs=1) as wadp:
                bg_row = wadp.tile([1, 4 * 384], BF16, name="bg_row")
                dma(out=bg_row[:], in_=t["bad2g"][:])
                sh_row = wadp.tile([1, 4 * 384], F32, name="sh_row")
                for half in range(4):
                    wad2 = wadp.tile(
                        [P, NDC, 2 * 384], BF16, tag="wad2", name=f"wad2_{half}"
                    )
                    dma2(
                        out=wad2[:],
                        in_=t["wad2"][:, :, half * 768 : (half + 1) * 768],
                    )
                    for t2 in range(2):
                        tt = half * 2 + t2
                        ps = psS.tile([P, 1024], F32, tag="ps_s", name=f"psa2_{tt}")
                        for dc in range(NDC):
                            mm(
                                ps[0:1, 0:384],
                                sc_bf[:, dc : dc + 1],
                                wad2[:, dc, t2 * 384 : (t2 + 1) * 384],
                                start=(dc == 0),
                                stop=(dc == NDC - 1),
                            )
                        if tt < 4:
                            # sh2|sc2: copy to an SBUF row (DMA can't read
                            # PSUM), then bounce through DRAM into columns
                            v.tensor_copy(
                                sh_row[:, tt * 384 : (tt + 1) * 384],
                                ps[0:1, 0:384],
                            )
                        else:
                            # g1|g2: copy to SBUF row, add bias, broadcast
                            g_sb = wadp.tile(
                                [1, 384], F32, tag="g_sb", name=f"g_sb{tt}"
                            )
                            v.tensor_copy(g_sb[:], ps[0:1, 0:384])
                            gi = tt - 4
                            v.tensor_add(
                                g_sb[:], g_sb[:],
                                bg_row[:, gi * 384 : (gi + 1) * 384],
                            )
                            dst = g1s_b if gi < 2 else g2s_b
                            col = (gi % 2) * 384
                            gp.partition_broadcast(
                                dst[:, col : col + 384], g_sb[:]
                            )
                dma(out=drB[:].rearrange("(o d) -> o d", o=1), in_=sh_row[:])
                sh2_raw = wadp.tile([P, NDC], F32, name="sh2_raw")
                dma(out=sh2_raw[:], in_=drB[0:D].rearrange("(c p) -> p c", p=P))
                sp2_raw = wadp.tile([P, NDC], F32, name="sp2_raw")
                dma(
                    out=sp2_raw[:],
                    in_=drB[D : 2 * D].rearrange("(c p) -> p c", p=P),
                )
                b2sh = wadp.tile([P, NDC], F32, name="b2sh")
                dma(out=b2sh[:], in_=t["bad2sh"][:, 0:NDC])
                b2sc = wadp.tile([P, NDC], F32, name="b2sc")
                dma(out=b2sc[:], in_=t["bad2sh"][:, NDC : 2 * NDC])
                v.tensor_add(sh2_col[:], sh2_raw[:], b2sh[:])
                v.tensor_add(sp2_col[:], sp2_raw[:], b2sc[:])

        emit_K(0)
        NK2 = NKC // 2
        for hp in range(NHP):
            ps_o = [
                psO.tile([VE, LQ], F32, tag=f"ps_o{dlt}", name=f"psO{hp}_{dlt}")
                for dlt in range(2)
            ]
            pending = None
            for kc2 in range(NK2):
                ps_s = [
                    psS.tile([P, 1024], F32, tag="ps_s", name=f"ps_s{_d}")
                    for _d in range(2)
                ]
                for j in range(2):
                    kc = 2 * kc2 + j
                    for dlt in range(2):
                        off = dlt * HD
                        mm(
                            ps_s[dlt][:, j * 512 : (j + 1) * 512],
                            kT_all[off : off + HD, hp, kc * P : (kc + 1) * P],
                            qT_all[off : off + HD, hp, :],
                            start=True,
                            stop=True,
                        )
                pts = []
                for dlt in range(2):
                    ptile = pt_pool.tile([P, 1024], FP8E5, tag="pt", name=f"pt{dlt}")
                    if dlt == 0 or kc2 % 8 == 0:
                        act(ptile[:], ps_s[dlt][:], AF.Exp, scale=0.125)
                    else:
                        v.tensor_scalar(
                            ptile.bitcast(I8)[:], ps_s[dlt][:], SCH_A, SCH_B,
                            op0=OP.mult, op1=OP.add,
                        )
                    pts.append(ptile)
                if pending is not None:
                    pk2, ppts = pending
                    for dlt in range(2):
                        h = 2 * hp + dlt
                        mm(
                            ps_o[dlt][:],
                            v4[:, 2 * pk2 : 2 * pk2 + 2, h * VE : (h + 1) * VE],
                            ppts[dlt].rearrange("p (j n) -> p j n", j=2)[:],
                            start=(pk2 == 0),
                            stop=False,
                            perf_mode=DR,
                        )
                pending = (kc2, pts)
                if kc2 == 7 and hp + 1 < NHP:
                    emit_K(hp + 1)
                if kc2 == 11 and hp == 0:
                    emit_adaln2()
            pk2, ppts = pending
            for dlt in range(2):
                h = 2 * hp + dlt
                mm(
                    ps_o[dlt][:],
                    v4[:, 2 * pk2 : 2 * pk2 + 2, h * VE : (h + 1) * VE],
                    ppts[dlt].rearrange("p (j n) -> p j n", j=2)[:],
                    start=False,
                    stop=True,
                    perf_mode=DR,
                )
            for dlt in range(2):
                off = dlt * HD
                zr = tiny.tile([1, LQ], F32, tag="zr", name=f"zr{dlt}")
                v.tensor_copy(zr[:], ps_o[dlt][HD : HD + 1, :])
                rz_f = tiny.tile([1, LQ], F32, tag="rz_f", name=f"rz_f{dlt}")
                v.reciprocal_approx_fast(rz_f[:], zr[:])
                rz_bf = tiny.tile([1, LQ], BF16, tag="rz_bf", name=f"rz_bf{dlt}")
                v.tensor_copy(rz_bf[:], rz_f[:])
                rzb = tiny.tile([P, LQ], BF16, tag="rzb", name=f"rzb{dlt}")
                gp.partition_broadcast(rzb[:], rz_bf[:])
                v.tensor_tensor(
                    catT8[off : off + HD, hp, :],
                    ps_o[dlt][0:HD, :],
                    rzb[0:HD, :],
                    op=OP.mult,
                )

    s_attn.close()  # free K/V/Q/xn1T8 space before the FFN weights land

    # -------- phase D: out-projection, residual, LN2 ---------------------
    with ExitStack() as phD:
        pool = phD.enter_context(tc.tile_pool(name="phD", bufs=2))
        spool = phD.enter_context(tc.tile_pool(name="spoolE", bufs=4))
        tpool2 = phD.enter_context(tc.tile_pool(name="tpool2", bufs=1))
        psD1 = phD.enter_context(tc.tile_pool(name="psD1", bufs=2, space="PSUM"))
        psD2 = phD.enter_context(tc.tile_pool(name="psD2", bufs=2, space="PSUM"))

        w8ao = pool.tile([P, NDC, D], FP8, name="w8ao")
        dma(out=w8ao[:], in_=t["w_ao8"].rearrange("(c p) m -> p c m", p=P))
        ba_sb = pool.tile([P, D], F32, name="ba_sb")
        dma(out=ba_sb[:], in_=t["b_attn_b"][:])
        bf2_sb = pool.tile([P, D], F32, name="bf2_sb")
        dma(out=bf2_sb[:], in_=t["b_ffn2_b"][:])
        gp.tensor_tensor(xb_bias[:], ba_sb[:], g1s_b[:], op=OP.mult)
        gp.tensor_tensor(x2b_bias[:], bf2_sb[:], g2s_b[:], op=OP.mult)
        xbl = [pool.tile([P, D], F32, name=f"xbl{q}") for q in range(NQC)]
        for q in range(NQC):
            v.tensor_add(xbl[q][:], x_loc[:, q, :], xb_bias[:])

        xn2T_bf = tpool2.tile([P, NDC, LQ], BF16)
        for qc in range(NQC):
            ps1 = psD1.tile([P, 512], F32)
            ps2 = psD2.tile([P, 256], F32)
            for cc2 in range(NDC // 2):
                lhs = catT8[:, 2 * cc2 : 2 * cc2 + 2, qc * P : (qc + 1) * P]
                mm(ps1[:], lhs, w8ao[:, 2 * cc2 : 2 * cc2 + 2, 0:512],
                   start=(cc2 == 0), stop=(cc2 == NDC // 2 - 1), perf_mode=DR)
                mm(ps2[:], lhs, w8ao[:, 2 * cc2 : 2 * cc2 + 2, 512:D],
                   start=(cc2 == 0), stop=(cc2 == NDC // 2 - 1), perf_mode=DR)
            gt = pool.tile([P, D], F32, tag="gt", name=f"gt{qc}")
            v.scalar_tensor_tensor(
                gt[:, 0:512], ps1[:], 1.0 / (SW_AO * SW_CAT), g1s_b[:, 0:512],
                op0=OP.mult, op1=OP.mult,
            )
            v.scalar_tensor_tensor(
                gt[:, 512:D], ps2[:], 1.0 / (SW_AO * SW_CAT), g1s_b[:, 512:D],
                op0=OP.mult, op1=OP.mult,
            )
            v.tensor_add(x2_loc[qc][:], gt[:], xbl[qc][:])
        for qc in range(NQC):
            stats = spool.tile([P, 2, 6], F32, tag="st2")
            for g in range(2):
                v.bn_stats(stats[:, g, :], x2_loc[qc][:, g * 384 : (g + 1) * 384])
            mv = spool.tile([P, 2], F32, tag="mv2", name=f"mv2_{qc}")
            v.bn_aggr(mv[:], stats[:])
            sq = spool.tile([P, 1], F32, tag="sq2")
            act(sq[:], mv[:, 1:2], AF.Sqrt, bias=eps_t[:, 0:1])
            rstd = spool.tile([P, 1], F32, tag="rstd2", name=f"rstd2_{qc}")
            v.reciprocal_approx_fast(rstd[:], sq[:])
            nx = spool.tile([P, D], BF16, tag="nx2", name=f"nx2_{qc}")
            gp.tensor_scalar(
                nx[:], x2_loc[qc][:], mv[:, 0:1], rstd[:, 0:1],
                op0=OP.subtract, op1=OP.mult,
            )
            dmat(out=xn2T_bf[:, :, qc * P : (qc + 1) * P], in_=nx[:])
        for dc in range(NDC):
            e3 = dc % 3
            if e3 == 0:
                act(
                    xn2T8[:, dc, :], xn2T_bf[:, dc, :], AF.Identity,
                    bias=sh2_col[:, dc : dc + 1], scale=sp2_col[:, dc : dc + 1],
                )
            elif e3 == 1:
                v.tensor_scalar(
                    xn2T8[:, dc, :], xn2T_bf[:, dc, :],
                    sp2_col[:, dc : dc + 1], sh2_col[:, dc : dc + 1],
                    op0=OP.mult, op1=OP.add,
                )
            else:
                gp.tensor_scalar(
                    xn2T8[:, dc, :], xn2T_bf[:, dc, :],
                    sp2_col[:, dc : dc + 1], sh2_col[:, dc : dc + 1],
                    op0=OP.mult, op1=OP.add,
                )

    # ---------------- phase F: FFN + gate + residual -> output ------------
    with ExitStack() as phF:
        wpool = phF.enter_context(tc.tile_pool(name="wffn", bufs=1))
        hpool = phF.enter_context(tc.tile_pool(name="hT", bufs=1))
        pool = phF.enter_context(tc.tile_pool(name="phF", bufs=2))
        psF1 = phF.enter_context(tc.tile_pool(name="psF1", bufs=3, space="PSUM"))
        psF2 = phF.enter_context(tc.tile_pool(name="psF2", bufs=2, space="PSUM"))

        w8f1 = wpool.tile([P, NDC, DM], FP8)
        wr = t["w_ffn18"].rearrange("(c p) m -> p c m", p=P)
        for q4 in range(4):
            dma2(
                out=w8f1[:, :, q4 * D : (q4 + 1) * D],
                in_=wr[:, :, q4 * D : (q4 + 1) * D],
            )
        bf1_col = wpool.tile([P, NMC], F32)
        dma(out=bf1_col[:], in_=t["b_ffn1_col"][:])
        w8f2 = wpool.tile([P, NMC, D], FP8)
        dma2(out=w8f2[:], in_=t["w_f28"].rearrange("(c p) m -> p c m", p=P))
        for q in range(NQC):
            v.tensor_add(x2_loc[q][:], x2_loc[q][:], x2b_bias[:])

        hT8 = hpool.tile([P, NMC, LQ], FP8)
        for mc in range(NMC):
            ps_h = psF1.tile([P, 512], F32, tag="mm512")
            for dc2 in range(NDC // 2):
                mm(
                    ps_h[:],
                    w8f1[:, 2 * dc2 : 2 * dc2 + 2, mc * P : (mc + 1) * P],
                    xn2T8[:, 2 * dc2 : 2 * dc2 + 2, :],
                    start=(dc2 == 0),
                    stop=(dc2 == NDC // 2 - 1),
                    perf_mode=DR,
                )
            act(
                hT8[:, mc, :], ps_h[:], AF.Gelu,
                bias=bf1_col[:, mc : mc + 1], scale=1.0 / SW_F1,
            )

        out_r = t["out"].rearrange("(n p) d -> n p d", p=P)
        for qc in range(NQC):
            ps1 = psF1.tile([P, 512], F32, tag="mm512")
            ps2 = psF2.tile([P, 256], F32)
            for mc2 in range(NMC // 2):
                lhs = hT8[:, 2 * mc2 : 2 * mc2 + 2, qc * P : (qc + 1) * P]
                mm(ps1[:], lhs, w8f2[:, 2 * mc2 : 2 * mc2 + 2, 0:512],
                   start=(mc2 == 0), stop=(mc2 == NMC // 2 - 1), perf_mode=DR)
                mm(ps2[:], lhs, w8f2[:, 2 * mc2 : 2 * mc2 + 2, 512:D],
                   start=(mc2 == 0), stop=(mc2 == NMC // 2 - 1), perf_mode=DR)
            gt = pool.tile([P, D], F32, tag="gt")
            v.scalar_tensor_tensor(
                gt[:, 0:512], ps1[:], 1.0 / SW_F2, g2s_b[:, 0:512],
                op0=OP.mult, op1=OP.mult,
            )
            v.scalar_tensor_tensor(
                gt[:, 512:D], ps2[:], 1.0 / SW_F2, g2s_b[:, 512:D],
                op0=OP.mult, op1=OP.mult,
            )
            ot = pool.tile([P, D], F32)
            v.tensor_add(ot[:], gt[:], x2_loc[qc][:])
            dma(out=out_r[qc], in_=ot[:])


def build_nc():
    nc = bacc.Bacc(None, target_bir_lowering=False, debug=False)
    t = _declare_params(nc)
    with tile.TileContext(nc) as tc:
        with ExitStack() as ctx:
            _build_body(nc, tc, ctx, t)
    nc.compile()
    return nc


_cache = {}


def _prep_in_maps(inputs):
    E4 = ml_dtypes.float8_e4m3fn
    f32 = lambda a: np.ascontiguousarray(np.asarray(a, np.float32))
    q8 = lambda a, s: np.ascontiguousarray(
        (np.asarray(a, np.float32) * s).astype(E4)
    )
    x = f32(inputs["x"]).reshape(L, D)
    cond = f32(inputs["cond"]).reshape(D)
    b_qkv = f32(inputs["b_qkv"]).reshape(3 * D)
    w_ao = f32(inputs["w_attn_out"])
    b_attn_eff = f32(inputs["b_attn_out"]).reshape(D) + b_qkv[2 * D :] @ w_ao
    w_ad1 = f32(inputs["w_adaln1"])  # [D, 3D]: sh1|sc1|g1
    w_ad2 = f32(inputs["w_adaln2"])
    b_ad1 = f32(inputs["b_adaln1"]).reshape(3 * D)
    b_ad2 = f32(inputs["b_adaln2"]).reshape(3 * D)
    wad1 = w_ad1[:, 0 : 2 * D]
    wad2 = np.concatenate(
        [w_ad2[:, 0 : 2 * D], w_ad1[:, 2 * D :], w_ad2[:, 2 * D :]], axis=1
    )
    bad1 = b_ad1[0 : 2 * D]
    b2shc = np.zeros((P, 2 * NDC), np.float32)
    b2shc[:, 0:NDC] = b_ad2[0:D].reshape(NDC, P).T
    b2shc[:, NDC : 2 * NDC] = b_ad2[D : 2 * D].reshape(NDC, P).T + 1.0
    bad2g = np.concatenate([b_ad1[2 * D :], b_ad2[2 * D :]])
    common = {
        "cond_t": np.ascontiguousarray(cond.reshape(NDC, P).T),
        "wad1": np.ascontiguousarray(
            wad1.reshape(NDC, P, 2 * D).transpose(1, 0, 2)
        ).astype(ml_dtypes.bfloat16),
        "wad2": np.ascontiguousarray(
            wad2.reshape(NDC, P, 4 * D).transpose(1, 0, 2)
        ).astype(ml_dtypes.bfloat16),
        "bad1": np.ascontiguousarray(bad1[None]),
        "bad2sh": np.ascontiguousarray(b2shc),
        "bad2g": np.ascontiguousarray(bad2g[None]).astype(ml_dtypes.bfloat16),
        "w_qkv8": q8(inputs["w_qkv"], SW_QKV),
        "b_q_col": np.ascontiguousarray(b_qkv[:D].reshape(NDC, P).T),
        "w_ao8": q8(w_ao, SW_AO),
        "b_attn_b": np.ascontiguousarray(np.broadcast_to(b_attn_eff, (P, D))),
        "w_ffn18": q8(inputs["w_ffn1"], SW_F1),
        "b_ffn1_col": np.ascontiguousarray(
            f32(inputs["b_ffn1"]).reshape(NMC, P).T
        ),
        "w_f28": q8(inputs["w_ffn2"], SW_F2),
        "b_ffn2_b": np.ascontiguousarray(
            np.broadcast_to(f32(inputs["b_ffn2"]).reshape(D), (P, D))
        ),
    }
    in_maps = []
    for c in range(NCORES):
        m = dict(common)
        xr = np.roll(x, -c * LQ, axis=0)
        m["x"] = np.ascontiguousarray(xr[:LQ])
        m["x_bf"] = np.ascontiguousarray(xr.astype(ml_dtypes.bfloat16))
        in_maps.append(m)
    return in_maps


def kernel(**inputs):
    if "nc" not in _cache:
        _cache["nc"] = build_nc()
    nc = _cache["nc"]
    in_maps = _prep_in_maps(inputs)
    res = run_bass_kernel_spmd(nc, in_maps, list(range(NCORES)))
    out = np.concatenate([res.results[c]["out"] for c in range(NCORES)], axis=0)
    return out.reshape(1, L, D).astype(np.float32)


if __name__ == "__main__":
    rng = np.random.default_rng(0)
    fake = {
        "x": rng.standard_normal((1, L, D), dtype=np.float32),
        "cond": rng.standard_normal((1, D), dtype=np.float32),
        "w_adaln1": rng.standard_normal((D, 3 * D), dtype=np.float32) * 0.02,
        "b_adaln1": np.zeros(3 * D, np.float32),
        "w_qkv": rng.standard_normal((D, 3 * D), dtype=np.float32) * D**-0.5,
        "b_qkv": np.zeros(3 * D, np.float32),
        "w_attn_out": rng.standard_normal((D, D), dtype=np.float32) * D**-0.5,
        "b_attn_out": np.zeros(D, np.float32),
        "w_adaln2": rng.standard_normal((D, 3 * D), dtype=np.float32) * 0.02,
        "b_adaln2": np.zeros(3 * D, np.float32),
        "w_ffn1": rng.standard_normal((D, DM), dtype=np.float32) * D**-0.5,
        "b_ffn1": np.zeros(DM, np.float32),
        "w_ffn2": rng.standard_normal((DM, D), dtype=np.float32) * DM**-0.5,
        "b_ffn2": np.zeros(D, np.float32),
    }
    out = kernel(**fake)
    print("out", out.shape, out.dtype, np.abs(out).max())


# revision 20
# speedup vs baseline: 1.5236x; 1.5236x over previous
"""DiT block kernel for 8 Trainium2 NeuronCores (Bass/Tile).

Sharding: sequence dim L=4096 split 8 ways (512 query rows per core).
Each core computes K/V for the full sequence (replicated compute; on
this stack a single cross-core collective costs ~50us of latency, more
than the ~60us of fully-pipelined extra matmul work it would save, so
no collectives are used). Inputs are rotated host-side so every core's
local rows sit at positions [0, 512) -> one SPMD program.

Structure: the LN1 -> xn1T8 -> V projection runs as a stream over 8
sequence blocks (VectorE stats, Pool normalize, DMA transposes, 3-way
modulate split); Q and K0 follow; K for pair hp+1 is emitted inside
attention pair hp where the PE has slack (attention is exp-bound).
Softmax exp alternates ScalarE (native Exp) / VectorE (Schraudolph
int8-bitcast) -- the only two engines that can read PSUM. AdaLN runs
in transposed orientation (few wide matmuls instead of many tiny
ones), shift/scale-1 up front, the rest hidden inside attention.

Precision: fp8e4 (e4m3) DoubleRow matmuls for QKV/out-proj/FFN; bf16
scores with the two heads of a pair on disjoint PE row-groups
(concurrent); fp8 AV (DoubleRow over key-chunk pairs); fp32 PSUM
accumulation; fp32 layernorm stats and residuals; attention output
scaled by 16 before the fp8 cast (undone in the out-projection).
"""

import sys

sys.path.insert(0, "/opt/trn_rl_repo")

from contextlib import ExitStack

import numpy as np
import ml_dtypes

import concourse.bass as bass
import concourse.bacc as bacc
import concourse.tile as tile
import concourse.mybir as mybir
from concourse.bass_utils import run_bass_kernel_spmd

F32 = mybir.dt.float32
BF16 = mybir.dt.bfloat16
FP8 = mybir.dt.float8e4
FP8E5 = mybir.dt.float8e5
I8 = mybir.dt.int8
AF = mybir.ActivationFunctionType
OP = mybir.AluOpType
DR = mybir.MatmulPerfMode.DoubleRow

L, D, H, HD, DM = 4096, 768, 12, 64, 3072
NCORES = 8
LQ = L // NCORES  # 512 local query rows
P = 128
EPS = 1e-5
NKC = L // P  # 32 k-chunks of 128
NQC = LQ // P  # 4 local q-chunks of 128
NDC = D // P  # 6 chunks of the model dim
NHP = H // 2  # 6 head pairs
NMC = DM // P  # 24 chunks of the FFN hidden dim
NBL = L // LQ  # 8 sequence blocks of 512
VE = HD + 4  # V row stride (64 dims + ones + pad)

SW_QKV = 64.0
SW_AO = 16.0
SW_F1 = 16.0
SW_F2 = 32.0
SW_CAT = 16.0  # ones-row = 1/SW_CAT so cat lands in fp8e4's sweet spot

# Schraudolph exp -> fp8e5 bits: i8 = round(x*2^2/ln2*0.125 + C)
SCH_A = 5.770780 * 0.125
SCH_B = 59.77


def _declare_params(nc):
    dp = nc.declare_dram_parameter
    t = {}
    t["x"] = dp("x", [LQ, D], F32, isOutput=False)
    t["x_bf"] = dp("x_bf", [L, D], BF16, isOutput=False)
    t["cond_t"] = dp("cond_t", [P, NDC], F32, isOutput=False)
    t["wad1"] = dp("wad1", [P, NDC, 2 * D], BF16, isOutput=False)  # sh1|sc1 cols
    t["wad2"] = dp("wad2", [P, NDC, 4 * D], BF16, isOutput=False)  # sh2|sc2|g1|g2
    t["bad1"] = dp("bad1", [1, 2 * D], F32, isOutput=False)
    t["bad2sh"] = dp("bad2sh", [P, 2 * NDC], F32, isOutput=False)
    t["bad2g"] = dp("bad2g", [1, 4 * 384], BF16, isOutput=False)
    t["w_qkv8"] = dp("w_qkv8", [D, 3 * D], FP8, isOutput=False)  # * SW_QKV
    t["b_q_col"] = dp("b_q_col", [P, NDC], F32, isOutput=False)
    t["w_ao8"] = dp("w_ao8", [D, D], FP8, isOutput=False)  # * SW_AO
    t["b_attn_b"] = dp("b_attn_b", [P, D], F32, isOutput=False)  # b_attn + bv@Wao
    t["w_ffn18"] = dp("w_ffn18", [D, DM], FP8, isOutput=False)  # * SW_F1
    t["b_ffn1_col"] = dp("b_ffn1_col", [P, NMC], F32, isOutput=False)
    t["w_f28"] = dp("w_f28", [DM, D], FP8, isOutput=False)  # * SW_F2
    t["b_ffn2_b"] = dp("b_ffn2_b", [P, D], F32, isOutput=False)
    t["out"] = dp("out", [LQ, D], F32, isOutput=True)
    return t


def _build_body(nc, tc, ctx, t):
    mm = nc.tensor.matmul
    dma = nc.sync.dma_start
    dma2 = nc.scalar.dma_start  # second HWDGE queue
    dmat = nc.sync.dma_start_transpose
    v = nc.vector
    gp = nc.gpsimd
    act = nc.scalar.activation

    const = ctx.enter_context(tc.tile_pool(name="const", bufs=1))
    eps_t = const.tile([P, 1], F32)
    v.memset(eps_t, EPS)

    dram = ctx.enter_context(tc.tile_pool(name="dram", bufs=1, space="DRAM"))
    drA = dram.tile([2 * D], F32)  # adaln1 row bounce
    drB = dram.tile([2 * D], F32)  # adaln2 sh/sc row bounce

    adaln = ctx.enter_context(tc.tile_pool(name="adaln", bufs=1))
    sh1_col = adaln.tile([P, NDC], F32)
    sp1_col = adaln.tile([P, NDC], F32)
    sh2_col = adaln.tile([P, NDC], F32)
    sp2_col = adaln.tile([P, NDC], F32)
    g1s_b = adaln.tile([P, D], F32)
    g2s_b = adaln.tile([P, D], F32)
    xb_bias = adaln.tile([P, D], F32)
    x2b_bias = adaln.tile([P, D], F32)
    sc_bf = adaln.tile([P, NDC], BF16)

    # ---------------- phase A: adaln1 (sh1/sc1 only) ----------------------
    with ExitStack() as phA:
        pool = phA.enter_context(tc.tile_pool(name="phA", bufs=1))
        psA = phA.enter_context(tc.tile_pool(name="psA", bufs=2, space="PSUM"))

        cond_sb = pool.tile([P, NDC], F32)
        dma(out=cond_sb[:], in_=t["cond_t"][:])
        sc_f = pool.tile([P, NDC], F32)
        act(sc_f[:], cond_sb[:], AF.Silu)
        v.tensor_copy(sc_bf[:], sc_f[:])

        wad1 = pool.tile([P, NDC, 2 * D], BF16)
        dma2(out=wad1[:], in_=t["wad1"][:])
        b1row = pool.tile([1, 2 * D], F32)
        dma(out=b1row[:], in_=t["bad1"][:])

        a1row = pool.tile([1, 2 * D], F32)
        for tt in range(3):
            ps = psA.tile([1, 512], F32)
            for dc in range(NDC):
                mm(
                    ps[:],
                    sc_bf[:, dc : dc + 1],
                    wad1[:, dc, tt * 512 : (tt + 1) * 512],
                    start=(dc == 0),
                    stop=(dc == NDC - 1),
                )
            v.tensor_add(
                a1row[:, tt * 512 : (tt + 1) * 512], ps[:],
                b1row[:, tt * 512 : (tt + 1) * 512],
            )
        dma(out=drA[:].rearrange("(o d) -> o d", o=1), in_=a1row[:])
        dma(out=sh1_col[:], in_=drA[0:D].rearrange("(c p) -> p c", p=P))
        sp1_raw = pool.tile([P, NDC], F32)
        dma(out=sp1_raw[:], in_=drA[D : 2 * D].rearrange("(c p) -> p c", p=P))
        v.tensor_scalar_add(sp1_col[:], sp1_raw[:], 1.0)

    # ---------------- phase B: streamed LN1 + V/Q -------------------------
    big = ctx.enter_context(tc.tile_pool(name="big", bufs=1))
    x_loc = big.tile([P, NQC, D], F32)
    x2_loc = [big.tile([P, D], F32, name=f"x2_loc{q}") for q in range(NQC)]
    catT8 = big.tile([P, NDC, LQ], FP8)
    xn2T8 = big.tile([P, NDC, LQ], FP8)

    s_attn = ctx.enter_context(ExitStack())
    attn_pool = s_attn.enter_context(tc.tile_pool(name="attn", bufs=1))
    kT_all = attn_pool.tile([P, NHP, L], BF16)
    v4 = attn_pool.tile([P, NKC, H * VE], FP8)
    qT_all = attn_pool.tile([P, NHP, LQ], BF16)
    xn1T8 = [
        attn_pool.tile([P, NDC, LQ], FP8, name=f"xn1T8_{b}") for b in range(NBL)
    ]
    w8qkv = attn_pool.tile([P, NDC, 3 * D], FP8)
    bq_col = attn_pool.tile([P, NDC], F32)

    dma(out=x_loc[:], in_=t["x"].rearrange("(n p) d -> p n d", p=P))
    dma2(out=w8qkv[:], in_=t["w_qkv8"].rearrange("(c p) m -> p c m", p=P))
    dma(out=bq_col[:], in_=t["b_q_col"][:])
    v4r = v4.rearrange("p k (h e) -> p k h e", e=VE)
    v.memset(v4r[:, :, :, HD : HD + 1], 1.0 / SW_CAT)
    v.memset(v4r[:, :, :, HD + 1 :], 0.0)

    phB = ctx.enter_context(ExitStack())
    xload = phB.enter_context(tc.tile_pool(name="xload", bufs=10))
    spool = phB.enter_context(tc.tile_pool(name="spool", bufs=10))
    nxpool = phB.enter_context(tc.tile_pool(name="nxpool", bufs=8))
    tpool = phB.enter_context(tc.tile_pool(name="tpool", bufs=2))
    psB1 = phB.enter_context(tc.tile_pool(name="psB1", bufs=2, space="PSUM"))
    psB2 = phB.enter_context(tc.tile_pool(name="psB2", bufs=2, space="PSUM"))

    x_r = t["x_bf"].rearrange("(n p) d -> n p d", p=P)
    for b in range(NBL):
        xn1T_bf = tpool.tile([P, NDC, LQ], BF16, tag="xnbf", name=f"xnbf{b}")
        nxs = []
        for ii in range(4):
            i = b * 4 + ii
            xt = xload.tile([P, D], BF16, tag="xt", name=f"xt{ii}")
            dma(out=xt[:], in_=x_r[i])
            stats = spool.tile([P, 2, 6], F32, tag="stats", name=f"st{ii}")
            for g in range(2):
                v.bn_stats(stats[:, g, :], xt[:, g * 384 : (g + 1) * 384])
            mv = spool.tile([P, 2], F32, tag="mv", name=f"mv{ii}")
            v.bn_aggr(mv[:], stats[:])
            sq = spool.tile([P, 1], F32, tag="sq", name=f"sq{ii}")
            act(sq[:], mv[:, 1:2], AF.Sqrt, bias=eps_t[:, 0:1])
            rstd = spool.tile([P, 1], F32, tag="rstd", name=f"rstd{ii}")
            v.reciprocal_approx_fast(rstd[:], sq[:])
            nx = nxpool.tile([P, D], BF16, tag="nx", name=f"nx{ii}")
            v.tensor_scalar(
                nx[:], xt[:], mv[:, 0:1], rstd[:, 0:1],
                op0=OP.subtract, op1=OP.mult,
            )
            nxs.append(nx)
        for ii in range(4):
            dmat(out=xn1T_bf[:, :, ii * P : (ii + 1) * P], in_=nxs[ii][:])
        for dc in range(NDC):
            if dc % 2 == 0:
                act(
                    xn1T8[b][:, dc, :], xn1T_bf[:, dc, :], AF.Identity,
                    bias=sh1_col[:, dc : dc + 1], scale=sp1_col[:, dc : dc + 1],
                )
            else:
                v.tensor_scalar(
                    xn1T8[b][:, dc, :], xn1T_bf[:, dc, :],
                    sp1_col[:, dc : dc + 1], sh1_col[:, dc : dc + 1],
                    op0=OP.mult, op1=OP.add,
                )
        # V projection for this block (4 k-chunks)
        for lc in range(4):
            ps_v = psB2.tile([P, D], F32, tag="psv", name=f"psv{lc}")
            for dc2 in range(NDC // 2):
                lhs = xn1T8[b][:, 2 * dc2 : 2 * dc2 + 2, lc * P : (lc + 1) * P]
                mm(ps_v[:, 0:512], lhs,
                   w8qkv[:, 2 * dc2 : 2 * dc2 + 2, 2 * D : 2 * D + 512],
                   start=(dc2 == 0), stop=(dc2 == NDC // 2 - 1), perf_mode=DR)
                mm(ps_v[:, 512:D], lhs,
                   w8qkv[:, 2 * dc2 : 2 * dc2 + 2, 2 * D + 512 : 3 * D],
                   start=(dc2 == 0), stop=(dc2 == NDC // 2 - 1), perf_mode=DR)
            kc = b * 4 + lc
            if lc % 2 == 0:
                act(v4r[:, kc, :, 0:HD],
                    ps_v.rearrange("p (h e) -> p h e", e=HD),
                    AF.Copy, scale=1.0 / SW_QKV)
            else:
                v.tensor_scalar(
                    v4r[:, kc, :, 0:HD],
                    ps_v.rearrange("p (h e) -> p h e", e=HD),
                    1.0 / SW_QKV, 0.0, op0=OP.mult, op1=OP.add,
                )
        if b == 0:
            # Q projection (local rows = block 0 thanks to the roll)
            for hp in range(NHP):
                ps_q = psB1.tile([P, LQ], F32, tag="psq", name=f"psq{hp}")
                for dc2 in range(NDC // 2):
                    mm(
                        ps_q[:],
                        w8qkv[:, 2 * dc2 : 2 * dc2 + 2, hp * P : (hp + 1) * P],
                        xn1T8[0][:, 2 * dc2 : 2 * dc2 + 2, :],
                        start=(dc2 == 0),
                        stop=(dc2 == NDC // 2 - 1),
                        perf_mode=DR,
                    )
                v.tensor_scalar(
                    qT_all[:, hp, :], ps_q[:], 1.0 / SW_QKV,
                    bq_col[:, hp : hp + 1], op0=OP.mult, op1=OP.add,
                )

    phB.close()  # release LN1 streaming pools before attention
    # ---------------- phase C: attention (+K emit, +adaln2) ---------------
    with ExitStack() as phC:
        pt_pool = phC.enter_context(tc.tile_pool(name="ptp", bufs=6))
        tiny = phC.enter_context(tc.tile_pool(name="tiny", bufs=2))
        psS = phC.enter_context(tc.tile_pool(name="psS", bufs=2, space="PSUM"))
        psK = phC.enter_context(tc.tile_pool(name="psK", bufs=2, space="PSUM"))
        psO = phC.enter_context(tc.tile_pool(name="psO", bufs=1, space="PSUM"))

        def emit_K(hp):
            # K for head pair hp over the full sequence; bias dropped
            # (constant per query -> softmax invariant), 1/SW at convert.
            for b in range(NBL):
                ps_k = psK.tile([P, LQ], F32, tag="ps_k", name=f"psk{b}")
                for dc2 in range(NDC // 2):
                    mm(
                        ps_k[:],
                        w8qkv[:, 2 * dc2 : 2 * dc2 + 2, D + hp * P : D + (hp + 1) * P],
                        xn1T8[b][:, 2 * dc2 : 2 * dc2 + 2, :],
                        start=(dc2 == 0),
                        stop=(dc2 == NDC // 2 - 1),
                        perf_mode=DR,
                    )
                act(kT_all[:, hp, b * LQ : (b + 1) * LQ], ps_k[:],
                    AF.Copy, scale=1.0 / SW_QKV)

        def emit_adaln2():
            # 8 tiles of 384 cols aligned to [sh2|sc2|g1|g2] groups
            with tc.tile_pool(name="wadp", bufs=1) as wadp:
                bg_row = wadp.tile([1, 4 * 384], BF16, name="bg_row")
                dma(out=bg_row[:], in_=t["bad2g"][:])
                sh_row = wadp.tile([1, 4 * 384], F32, name="sh_row")
                for half in range(4):
                    wad2 = wadp.tile(
                        [P, NDC, 2 * 384], BF16, tag="wad2", name=f"wad2_{half}"
                    )
                    dma2(
                        out=wad2[:],
                        in_=t["wad2"][:, :, half * 768 : (half + 1) * 768],
                    )
                    for t2 in range(2):
                        tt = half * 2 + t2
                        ps = psS.tile([P, 1024], F32, tag="ps_s", name=f"psa2_{tt}")
                        for dc in range(NDC):
                            mm(
                                ps[0:1, 0:384],
                                sc_bf[:, dc : dc + 1],
                                wad2[:, dc, t2 * 384 : (t2 + 1) * 384],
                                start=(dc == 0),
                                stop=(dc == NDC - 1),
                            )
                        if tt < 4:
                            # sh2|sc2: copy to an SBUF row (DMA can't read
                            # PSUM), then bounce through DRAM into columns
                            v.tensor_copy(
                                sh_row[:, tt * 384 : (tt + 1) * 384],
                                ps[0:1, 0:384],
                            )
                        else:
                            # g1|g2: copy to SBUF row, add bias, broadcast
                            g_sb = wadp.tile(
                                [1, 384], F32, tag="g_sb", name=f"g_sb{tt}"
                            )
                            v.tensor_copy(g_sb[:], ps[0:1, 0:384])
                            gi = tt - 4
                            v.tensor_add(
                                g_sb[:], g_sb[:],
                                bg_row[:, gi * 384 : (gi + 1) * 384],
                            )
                            dst = g1s_b if gi < 2 else g2s_b
                            col = (gi % 2) * 384
                            gp.partition_broadcast(
                                dst[:, col : col + 384], g_sb[:]
                            )
                dma(out=drB[:].rearrange("(o d) -> o d", o=1), in_=sh_row[:])
                sh2_raw = wadp.tile([P, NDC], F32, name="sh2_raw")
                dma(out=sh2_raw[:], in_=drB[0:D].rearrange("(c p) -> p c", p=P))
                sp2_raw = wadp.tile([P, NDC], F32, name="sp2_raw")
                dma(
                    out=sp2_raw[:],
                    in_=drB[D : 2 * D].rearrange("(c p) -> p c", p=P),
                )
                b2sh = wadp.tile([P, NDC], F32, name="b2sh")
                dma(out=b2sh[:], in_=t["bad2sh"][:, 0:NDC])
                b2sc = wadp.tile([P, NDC], F32, name="b2sc")
                dma(out=b2sc[:], in_=t["bad2sh"][:, NDC : 2 * NDC])
                v.tensor_add(sh2_col[:], sh2_raw[:], b2sh[:])
                v.tensor_add(sp2_col[:], sp2_raw[:], b2sc[:])

        emit_K(0)
        NK2 = NKC // 2
        for hp in range(NHP):
            ps_o = [
                psO.tile([VE, LQ], F32, tag=f"ps_o{dlt}", name=f"psO{hp}_{dlt}")
                for dlt in range(2)
            ]
            pending = None
            for kc2 in range(NK2):
                ps_s = [
                    psS.tile([P, 1024], F32, tag="ps_s", name=f"ps_s{_d}")
                    for _d in range(2)
                ]
                for j in range(2):
                    kc = 2 * kc2 + j
                    for dlt in range(2):
                        off = dlt * HD
                        mm(
                            ps_s[dlt][:, j * 512 : (j + 1) * 512],
                            kT_all[off : off + HD, hp, kc * P : (kc + 1) * P],
                            qT_all[off : off + HD, hp, :],
                            start=True,
                            stop=True,
                        )
                pts = []
                for dlt in range(2):
                    ptile = pt_pool.tile([P, 1024], FP8E5, tag="pt", name=f"pt{dlt}")
                    if dlt == 0 or kc2 % 8 == 0:
                        act(ptile[:], ps_s[dlt][:], AF.Exp, scale=0.125)
                    else:
                        v.tensor_scalar(
                            ptile.bitcast(I8)[:], ps_s[dlt][:], SCH_A, SCH_B,
                            op0=OP.mult, op1=OP.add,
                        )
                    pts.append(ptile)
                if pending is not None:
                    pk2, ppts = pending
                    for dlt in range(2):
                        h = 2 * hp + dlt
                        mm(
                            ps_o[dlt][:],
                            v4[:, 2 * pk2 : 2 * pk2 + 2, h * VE : (h + 1) * VE],
                            ppts[dlt].rearrange("p (j n) -> p j n", j=2)[:],
                            start=(pk2 == 0),
                            stop=False,
                            perf_mode=DR,
                        )
                pending = (kc2, pts)
                if kc2 == 7 and hp + 1 < NHP:
                    emit_K(hp + 1)
                if kc2 == 11 and hp == 0:
                    emit_adaln2()
            pk2, ppts = pending
            for dlt in range(2):
                h = 2 * hp + dlt
                mm(
                    ps_o[dlt][:],
                    v4[:, 2 * pk2 : 2 * pk2 + 2, h * VE : (h + 1) * VE],
                    ppts[dlt].rearrange("p (j n) -> p j n", j=2)[:],
                    start=False,
                    stop=True,
                    perf_mode=DR,
                )
            for dlt in range(2):
                off = dlt * HD
                zr = tiny.tile([1, LQ], F32, tag="zr", name=f"zr{dlt}")
                v.tensor_copy(zr[:], ps_o[dlt][HD : HD + 1, :])
                rz_f = tiny.tile([1, LQ], F32, tag="rz_f", name=f"rz_f{dlt}")
                v.reciprocal_approx_fast(rz_f[:], zr[:])
                rz_bf = tiny.tile([1, LQ], BF16, tag="rz_bf", name=f"rz_bf{dlt}")
                v.tensor_copy(rz_bf[:], rz_f[:])
                rzb = tiny.tile([P, LQ], BF16, tag="rzb", name=f"rzb{dlt}")
                gp.partition_broadcast(rzb[:], rz_bf[:])
                v.tensor_tensor(
                    catT8[off : off + HD, hp, :],
                    ps_o[dlt][0:HD, :],
                    rzb[0:HD, :],
                    op=OP.mult,
                )

    s_attn.close()  # free K/V/Q/xn1T8 space before the FFN weights land

    # -------- phase D: out-projection, residual, LN2 ---------------------
    with ExitStack() as phD:
        pool = phD.enter_context(tc.tile_pool(name="phD", bufs=2))
        spool = phD.enter_context(tc.tile_pool(name="spoolE", bufs=4))
        tpool2 = phD.enter_context(tc.tile_pool(name="tpool2", bufs=1))
        psD1 = phD.enter_context(tc.tile_pool(name="psD1", bufs=2, space="PSUM"))
        psD2 = phD.enter_context(tc.tile_pool(name="psD2", bufs=2, space="PSUM"))

        w8ao = pool.tile([P, NDC, D], FP8, name="w8ao")
        dma(out=w8ao[:], in_=t["w_ao8"].rearrange("(c p) m -> p c m", p=P))
        ba_sb = pool.tile([P, D], F32, name="ba_sb")
        dma(out=ba_sb[:], in_=t["b_attn_b"][:])
        bf2_sb = pool.tile([P, D], F32, name="bf2_sb")
        dma(out=bf2_sb[:], in_=t["b_ffn2_b"][:])
        v.tensor_tensor(xb_bias[:], ba_sb[:], g1s_b[:], op=OP.mult)
        v.tensor_tensor(x2b_bias[:], bf2_sb[:], g2s_b[:], op=OP.mult)
        xbl = [pool.tile([P, D], F32, name=f"xbl{q}") for q in range(NQC)]
        for q in range(NQC):
            v.tensor_add(xbl[q][:], x_loc[:, q, :], xb_bias[:])

        xn2T_bf = tpool2.tile([P, NDC, LQ], BF16)
        for qc in range(NQC):
            ps1 = psD1.tile([P, 512], F32)
            ps2 = psD2.tile([P, 256], F32)
            for cc2 in range(NDC // 2):
                lhs = catT8[:, 2 * cc2 : 2 * cc2 + 2, qc * P : (qc + 1) * P]
                mm(ps1[:], lhs, w8ao[:, 2 * cc2 : 2 * cc2 + 2, 0:512],
                   start=(cc2 == 0), stop=(cc2 == NDC // 2 - 1), perf_mode=DR)
                mm(ps2[:], lhs, w8ao[:, 2 * cc2 : 2 * cc2 + 2, 512:D],
                   start=(cc2 == 0), stop=(cc2 == NDC // 2 - 1), perf_mode=DR)
            gt = pool.tile([P, D], F32, tag="gt", name=f"gt{qc}")
            v.scalar_tensor_tensor(
                gt[:, 0:512], ps1[:], 1.0 / (SW_AO * SW_CAT), g1s_b[:, 0:512],
                op0=OP.mult, op1=OP.mult,
            )
            v.scalar_tensor_tensor(
                gt[:, 512:D], ps2[:], 1.0 / (SW_AO * SW_CAT), g1s_b[:, 512:D],
                op0=OP.mult, op1=OP.mult,
            )
            v.tensor_add(x2_loc[qc][:], gt[:], xbl[qc][:])
        for qc in range(NQC):
            stats = spool.tile([P, 2, 6], F32, tag="st2")
            for g in range(2):
                v.bn_stats(stats[:, g, :], x2_loc[qc][:, g * 384 : (g + 1) * 384])
            mv = spool.tile([P, 2], F32, tag="mv2", name=f"mv2_{qc}")
            v.bn_aggr(mv[:], stats[:])
            sq = spool.tile([P, 1], F32, tag="sq2")
            act(sq[:], mv[:, 1:2], AF.Sqrt, bias=eps_t[:, 0:1])
            rstd = spool.tile([P, 1], F32, tag="rstd2", name=f"rstd2_{qc}")
            v.reciprocal_approx_fast(rstd[:], sq[:])
            nx = spool.tile([P, D], BF16, tag="nx2", name=f"nx2_{qc}")
            v.tensor_scalar(
                nx[:], x2_loc[qc][:], mv[:, 0:1], rstd[:, 0:1],
                op0=OP.subtract, op1=OP.mult,
            )
            dmat(out=xn2T_bf[:, :, qc * P : (qc + 1) * P], in_=nx[:])
        for dc in range(NDC):
            if dc % 2 == 0:
                act(
                    xn2T8[:, dc, :], xn2T_bf[:, dc, :], AF.Identity,
                    bias=sh2_col[:, dc : dc + 1], scale=sp2_col[:, dc : dc + 1],
                )
            else:
                v.tensor_scalar(
                    xn2T8[:, dc, :], xn2T_bf[:, dc, :],
                    sp2_col[:, dc : dc + 1], sh2_col[:, dc : dc + 1],
                    op0=OP.mult, op1=OP.add,
                )

    # ---------------- phase F: FFN + gate + residual -> output ------------
    with ExitStack() as phF:
        wpool = phF.enter_context(tc.tile_pool(name="wffn", bufs=1))
        hpool = phF.enter_context(tc.tile_pool(name="hT", bufs=1))
        pool = phF.enter_context(tc.tile_pool(name="phF", bufs=2))
        psF1 = phF.enter_context(tc.tile_pool(name="psF1", bufs=3, space="PSUM"))
        psF2 = phF.enter_context(tc.tile_pool(name="psF2", bufs=2, space="PSUM"))

        w8f1 = wpool.tile([P, NDC, DM], FP8)
        wr = t["w_ffn18"].rearrange("(c p) m -> p c m", p=P)
        for q4 in range(4):
            dma2(
                out=w8f1[:, :, q4 * D : (q4 + 1) * D],
                in_=wr[:, :, q4 * D : (q4 + 1) * D],
            )
        bf1_col = wpool.tile([P, NMC], F32)
        dma(out=bf1_col[:], in_=t["b_ffn1_col"][:])
        w8f2 = wpool.tile([P, NMC, D], FP8)
        dma2(out=w8f2[:], in_=t["w_f28"].rearrange("(c p) m -> p c m", p=P))
        for q in range(NQC):
            v.tensor_add(x2_loc[q][:], x2_loc[q][:], x2b_bias[:])

        hT8 = hpool.tile([P, NMC, LQ], FP8)
        for mc in range(NMC):
            ps_h = psF1.tile([P, 512], F32, tag="mm512")
            for dc2 in range(NDC // 2):
                mm(
                    ps_h[:],
                    w8f1[:, 2 * dc2 : 2 * dc2 + 2, mc * P : (mc + 1) * P],
                    xn2T8[:, 2 * dc2 : 2 * dc2 + 2, :],
                    start=(dc2 == 0),
                    stop=(dc2 == NDC // 2 - 1),
                    perf_mode=DR,
                )
            act(
                hT8[:, mc, :], ps_h[:], AF.Gelu,
                bias=bf1_col[:, mc : mc + 1], scale=1.0 / SW_F1,
            )

        out_r = t["out"].rearrange("(n p) d -> n p d", p=P)
        for qc in range(NQC):
            ps1 = psF1.tile([P, 512], F32, tag="mm512")
            ps2 = psF2.tile([P, 256], F32)
            for mc2 in range(NMC // 2):
                lhs = hT8[:, 2 * mc2 : 2 * mc2 + 2, qc * P : (qc + 1) * P]
                mm(ps1[:], lhs, w8f2[:, 2 * mc2 : 2 * mc2 + 2, 0:512],
                   start=(mc2 == 0), stop=(mc2 == NMC // 2 - 1), perf_mode=DR)
                mm(ps2[:], lhs, w8f2[:, 2 * mc2 : 2 * mc2 + 2, 512:D],
                   start=(mc2 == 0), stop=(mc2 == NMC // 2 - 1), perf_mode=DR)
            gt = pool.tile([P, D], F32, tag="gt")
            v.scalar_tensor_tensor(
                gt[:, 0:512], ps1[:], 1.0 / SW_F2, g2s_b[:, 0:512],
                op0=OP.mult, op1=OP.mult,
            )
            v.scalar_tensor_tensor(
                gt[:, 512:D], ps2[:], 1.0 / SW_F2, g2s_b[:, 512:D],
                op0=OP.mult, op1=OP.mult,
            )
            ot = pool.tile([P, D], F32)
            v.tensor_add(ot[:], gt[:], x2_loc[qc][:])
            dma(out=out_r[qc], in_=ot[:])


def build_nc():
    nc = bacc.Bacc(None, target_bir_lowering=False, debug=False)
    t = _declare_params(nc)
    with tile.TileContext(nc) as tc:
        with ExitStack() as ctx:
            _build_body(nc, tc, ctx, t)
    nc.compile()
    return nc


_cache = {}


def _prep_in_maps(inputs):
    E4 = ml_dtypes.float8_e4m3fn
    f32 = lambda a: np.ascontiguousarray(np.asarray(a, np.float32))
    q8 = lambda a, s: np.ascontiguousarray(
        (np.asarray(a, np.float32) * s).astype(E4)
    )
    x = f32(inputs["x"]).reshape(L, D)
    cond = f32(inputs["cond"]).reshape(D)
    b_qkv = f32(inputs["b_qkv"]).reshape(3 * D)
    w_ao = f32(inputs["w_attn_out"])
    b_attn_eff = f32(inputs["b_attn_out"]).reshape(D) + b_qkv[2 * D :] @ w_ao
    w_ad1 = f32(inputs["w_adaln1"])  # [D, 3D]: sh1|sc1|g1
    w_ad2 = f32(inputs["w_adaln2"])
    b_ad1 = f32(inputs["b_adaln1"]).reshape(3 * D)
    b_ad2 = f32(inputs["b_adaln2"]).reshape(3 * D)
    wad1 = w_ad1[:, 0 : 2 * D]
    wad2 = np.concatenate(
        [w_ad2[:, 0 : 2 * D], w_ad1[:, 2 * D :], w_ad2[:, 2 * D :]], axis=1
    )
    bad1 = b_ad1[0 : 2 * D]
    b2shc = np.zeros((P, 2 * NDC), np.float32)
    b2shc[:, 0:NDC] = b_ad2[0:D].reshape(NDC, P).T
    b2shc[:, NDC : 2 * NDC] = b_ad2[D : 2 * D].reshape(NDC, P).T + 1.0
    bad2g = np.concatenate([b_ad1[2 * D :], b_ad2[2 * D :]])
    common = {
        "cond_t": np.ascontiguousarray(cond.reshape(NDC, P).T),
        "wad1": np.ascontiguousarray(
            wad1.reshape(NDC, P, 2 * D).transpose(1, 0, 2)
        ).astype(ml_dtypes.bfloat16),
        "wad2": np.ascontiguousarray(
            wad2.reshape(NDC, P, 4 * D).transpose(1, 0, 2)
        ).astype(ml_dtypes.bfloat16),
        "bad1": np.ascontiguousarray(bad1[None]),
        "bad2sh": np.ascontiguousarray(b2shc),
        "bad2g": np.ascontiguousarray(bad2g[None]).astype(ml_dtypes.bfloat16),
        "w_qkv8": q8(inputs["w_qkv"], SW_QKV),
        "b_q_col": np.ascontiguousarray(b_qkv[:D].reshape(NDC, P).T),
        "w_ao8": q8(w_ao, SW_AO),
        "b_attn_b": np.ascontiguousarray(np.broadcast_to(b_attn_eff, (P, D))),
        "w_ffn18": q8(inputs["w_ffn1"], SW_F1),
        "b_ffn1_col": np.ascontiguousarray(
            f32(inputs["b_ffn1"]).reshape(NMC, P).T
        ),
        "w_f28": q8(inputs["w_ffn2"], SW_F2),
        "b_ffn2_b": np.ascontiguousarray(
            np.broadcast_to(f32(inputs["b_ffn2"]).reshape(D), (P, D))
        ),
    }
    in_maps = []
    for c in range(NCORES):
        m = dict(common)
        xr = np.roll(x, -c * LQ, axis=0)
        m["x"] = np.ascontiguousarray(xr[:LQ])
        m["x_bf"] = np.ascontiguousarray(xr.astype(ml_dtypes.bfloat16))
        in_maps.append(m)
    return in_maps


def kernel(**inputs):
    if "nc" not in _cache:
        _cache["nc"] = build_nc()
    nc = _cache["nc"]
    in_maps = _prep_in_maps(inputs)
    res = run_bass_kernel_spmd(nc, in_maps, list(range(NCORES)))
    out = np.concatenate([res.results[c]["out"] for c in range(NCORES)], axis=0)
    return out.reshape(1, L, D).astype(np.float32)


if __name__ == "__main__":
    rng = np.random.default_rng(0)
    fake = {
        "x": rng.standard_normal((1, L, D), dtype=np.float32),
        "cond": rng.standard_normal((1, D), dtype=np.float32),
        "w_adaln1": rng.standard_normal((D, 3 * D), dtype=np.float32) * 0.02,
        "b_adaln1": np.zeros(3 * D, np.float32),
        "w_qkv": rng.standard_normal((D, 3 * D), dtype=np.float32) * D**-0.5,
        "b_qkv": np.zeros(3 * D, np.float32),
        "w_attn_out": rng.standard_normal((D, D), dtype=np.float32) * D**-0.5,
        "b_attn_out": np.zeros(D, np.float32),
        "w_adaln2": rng.standard_normal((D, 3 * D), dtype=np.float32) * 0.02,
        "b_adaln2": np.zeros(3 * D, np.float32),
        "w_ffn1": rng.standard_normal((D, DM), dtype=np.float32) * D**-0.5,
        "b_ffn1": np.zeros(DM, np.float32),
        "w_ffn2": rng.standard_normal((DM, D), dtype=np.float32) * DM**-0.5,
        "b_ffn2": np.zeros(D, np.float32),
    }
    out = kernel(**fake)
    print("out", out.shape, out.dtype, np.abs(out).max())


# revision 21
# speedup vs baseline: 1.6750x; 1.0994x over previous
"""DiT block kernel for 8 Trainium2 NeuronCores (Bass/Tile).

Sharding: sequence dim L=4096 split 8 ways (512 query rows per core).
Each core computes K/V for the full sequence (replicated compute; on
this stack a single cross-core collective costs ~50us of latency, more
than the ~60us of fully-pipelined extra matmul work it would save, so
no collectives are used). Inputs are rotated host-side so every core's
local rows sit at positions [0, 512) -> one SPMD program.

Structure: the LN1 -> xn1T8 -> V projection runs as a stream over 8
sequence blocks (VectorE stats, Pool normalize, DMA transposes, 3-way
modulate split); Q and K0 follow; K for pair hp+1 is emitted inside
attention pair hp where the PE has slack (attention is exp-bound).
Softmax exp alternates ScalarE (native Exp) / VectorE (Schraudolph
int8-bitcast) -- the only two engines that can read PSUM. AdaLN runs
in transposed orientation (few wide matmuls instead of many tiny
ones), shift/scale-1 up front, the rest hidden inside attention.

Precision: fp8e4 (e4m3) DoubleRow matmuls for QKV/out-proj/FFN; bf16
scores with the two heads of a pair on disjoint PE row-groups
(concurrent); fp8 AV (DoubleRow over key-chunk pairs); fp32 PSUM
accumulation; fp32 layernorm stats and residuals; attention output
scaled by 16 before the fp8 cast (undone in the out-projection).
"""

import sys

sys.path.insert(0, "/opt/trn_rl_repo")

from contextlib import ExitStack

import numpy as np
import ml_dtypes

import concourse.bass as bass
import concourse.bacc as bacc
import concourse.tile as tile
import concourse.mybir as mybir
from concourse.bass_utils import run_bass_kernel_spmd

F32 = mybir.dt.float32
BF16 = mybir.dt.bfloat16
FP8 = mybir.dt.float8e4
FP8E5 = mybir.dt.float8e5
I8 = mybir.dt.int8
AF = mybir.ActivationFunctionType
OP = mybir.AluOpType
DR = mybir.MatmulPerfMode.DoubleRow

L, D, H, HD, DM = 4096, 768, 12, 64, 3072
NCORES = 8
LQ = L // NCORES  # 512 local query rows
P = 128
EPS = 1e-5
NKC = L // P  # 32 k-chunks of 128
NQC = LQ // P  # 4 local q-chunks of 128
NDC = D // P  # 6 chunks of the model dim
NHP = H // 2  # 6 head pairs
NMC = DM // P  # 24 chunks of the FFN hidden dim
NBL = L // LQ  # 8 sequence blocks of 512
VE = HD + 4  # V row stride (64 dims + ones + pad)

SW_QKV = 64.0
SW_AO = 16.0
SW_F1 = 16.0
SW_F2 = 32.0
SW_CAT = 16.0  # ones-row = 1/SW_CAT so cat lands in fp8e4's sweet spot

# Schraudolph exp -> fp8e5 bits: i8 = round(x*2^2/ln2*0.125 + C)
SCH_A = 5.770780 * 0.125
SCH_B = 59.77


def _declare_params(nc):
    dp = nc.declare_dram_parameter
    t = {}
    t["x"] = dp("x", [LQ, D], F32, isOutput=False)
    t["x_bf"] = dp("x_bf", [L, D], BF16, isOutput=False)
    t["cond_t"] = dp("cond_t", [P, NDC], F32, isOutput=False)
    t["wad1"] = dp("wad1", [P, NDC, 2 * D], BF16, isOutput=False)  # sh1|sc1 cols
    t["wad2"] = dp("wad2", [P, NDC, 4 * D], BF16, isOutput=False)  # sh2|sc2|g1|g2
    t["bad1"] = dp("bad1", [1, 2 * D], F32, isOutput=False)
    t["bad2sh"] = dp("bad2sh", [P, 2 * NDC], F32, isOutput=False)
    t["bad2g"] = dp("bad2g", [1, 4 * 384], BF16, isOutput=False)
    t["w_qkv8"] = dp("w_qkv8", [D, 3 * D], FP8, isOutput=False)  # * SW_QKV
    t["b_q_col"] = dp("b_q_col", [P, NDC], F32, isOutput=False)
    t["w_ao8"] = dp("w_ao8", [D, D], FP8, isOutput=False)  # * SW_AO
    t["b_attn_b"] = dp("b_attn_b", [P, D], F32, isOutput=False)  # b_attn + bv@Wao
    t["w_ffn18"] = dp("w_ffn18", [D, DM], FP8, isOutput=False)  # * SW_F1
    t["b_ffn1_col"] = dp("b_ffn1_col", [P, NMC], F32, isOutput=False)
    t["w_f28"] = dp("w_f28", [DM, D], FP8, isOutput=False)  # * SW_F2
    t["b_ffn2_b"] = dp("b_ffn2_b", [P, D], F32, isOutput=False)
    t["out"] = dp("out", [LQ, D], F32, isOutput=True)
    return t


def _build_body(nc, tc, ctx, t):
    mm = nc.tensor.matmul
    dma = nc.sync.dma_start
    dma2 = nc.scalar.dma_start  # second HWDGE queue
    dmat = nc.sync.dma_start_transpose
    v = nc.vector
    gp = nc.gpsimd
    act = nc.scalar.activation

    const = ctx.enter_context(tc.tile_pool(name="const", bufs=1))
    eps_t = const.tile([P, 1], F32)
    v.memset(eps_t, EPS)

    dram = ctx.enter_context(tc.tile_pool(name="dram", bufs=1, space="DRAM"))
    drA = dram.tile([2 * D], F32)  # adaln1 row bounce
    drB = dram.tile([2 * D], F32)  # adaln2 sh/sc row bounce

    adaln = ctx.enter_context(tc.tile_pool(name="adaln", bufs=1))
    sh1_col = adaln.tile([P, NDC], F32)
    sp1_col = adaln.tile([P, NDC], F32)
    sh2_col = adaln.tile([P, NDC], F32)
    sp2_col = adaln.tile([P, NDC], F32)
    g1s_b = adaln.tile([P, D], F32)
    g2s_b = adaln.tile([P, D], F32)
    xb_bias = adaln.tile([P, D], F32)
    x2b_bias = adaln.tile([P, D], F32)
    sc_bf = adaln.tile([P, NDC], BF16)

    # ---------------- phase A: adaln1 (sh1/sc1 only) ----------------------
    with ExitStack() as phA:
        pool = phA.enter_context(tc.tile_pool(name="phA", bufs=1))
        psA = phA.enter_context(tc.tile_pool(name="psA", bufs=2, space="PSUM"))

        cond_sb = pool.tile([P, NDC], F32)
        dma(out=cond_sb[:], in_=t["cond_t"][:])
        sc_f = pool.tile([P, NDC], F32)
        act(sc_f[:], cond_sb[:], AF.Silu)
        v.tensor_copy(sc_bf[:], sc_f[:])

        wad1 = pool.tile([P, NDC, 2 * D], BF16)
        dma2(out=wad1[:], in_=t["wad1"][:])
        b1row = pool.tile([1, 2 * D], F32)
        dma(out=b1row[:], in_=t["bad1"][:])

        a1row = pool.tile([1, 2 * D], F32)
        for tt in range(3):
            ps = psA.tile([1, 512], F32)
            for dc in range(NDC):
                mm(
                    ps[:],
                    sc_bf[:, dc : dc + 1],
                    wad1[:, dc, tt * 512 : (tt + 1) * 512],
                    start=(dc == 0),
                    stop=(dc == NDC - 1),
                )
            v.tensor_add(
                a1row[:, tt * 512 : (tt + 1) * 512], ps[:],
                b1row[:, tt * 512 : (tt + 1) * 512],
            )
        dma(out=drA[:].rearrange("(o d) -> o d", o=1), in_=a1row[:])
        dma(out=sh1_col[:], in_=drA[0:D].rearrange("(c p) -> p c", p=P))
        sp1_raw = pool.tile([P, NDC], F32)
        dma(out=sp1_raw[:], in_=drA[D : 2 * D].rearrange("(c p) -> p c", p=P))
        v.tensor_scalar_add(sp1_col[:], sp1_raw[:], 1.0)

    # ---------------- phase B: streamed LN1 + V/Q -------------------------
    big = ctx.enter_context(tc.tile_pool(name="big", bufs=1))
    x_loc = big.tile([P, NQC, D], F32)
    x2_loc = [big.tile([P, D], F32, name=f"x2_loc{q}") for q in range(NQC)]
    catT8 = big.tile([P, NDC, LQ], FP8)
    xn2T8 = big.tile([P, NDC, LQ], FP8)

    s_attn = ctx.enter_context(ExitStack())
    attn_pool = s_attn.enter_context(tc.tile_pool(name="attn", bufs=1))
    kT_all = attn_pool.tile([P, NHP, L], BF16)
    v4 = attn_pool.tile([P, NKC, H * VE], FP8)
    qT_all = attn_pool.tile([P, NHP, LQ], BF16)
    xn1T8 = [
        attn_pool.tile([P, NDC, LQ], FP8, name=f"xn1T8_{b}") for b in range(NBL)
    ]
    w8qkv = attn_pool.tile([P, NDC, 3 * D], FP8)
    bq_col = attn_pool.tile([P, NDC], F32)

    dma(out=x_loc[:], in_=t["x"].rearrange("(n p) d -> p n d", p=P))
    dma2(out=w8qkv[:], in_=t["w_qkv8"].rearrange("(c p) m -> p c m", p=P))
    dma(out=bq_col[:], in_=t["b_q_col"][:])
    v4r = v4.rearrange("p k (h e) -> p k h e", e=VE)
    v.memset(v4r[:, :, :, HD : HD + 1], 1.0 / SW_CAT)
    v.memset(v4r[:, :, :, HD + 1 :], 0.0)

    phB = ctx.enter_context(ExitStack())
    xload = phB.enter_context(tc.tile_pool(name="xload", bufs=10))
    spool = phB.enter_context(tc.tile_pool(name="spool", bufs=10))
    nxpool = phB.enter_context(tc.tile_pool(name="nxpool", bufs=8))
    tpool = phB.enter_context(tc.tile_pool(name="tpool", bufs=2))
    psB1 = phB.enter_context(tc.tile_pool(name="psB1", bufs=2, space="PSUM"))
    psB2 = phB.enter_context(tc.tile_pool(name="psB2", bufs=2, space="PSUM"))

    x_r = t["x_bf"].rearrange("(n p) d -> n p d", p=P)
    for b in range(NBL):
        xn1T_bf = tpool.tile([P, NDC, LQ], BF16, tag="xnbf", name=f"xnbf{b}")
        nxs = []
        for ii in range(4):
            i = b * 4 + ii
            xt = xload.tile([P, D], BF16, tag="xt", name=f"xt{ii}")
            dma(out=xt[:], in_=x_r[i])
            stats = spool.tile([P, 2, 6], F32, tag="stats", name=f"st{ii}")
            for g in range(2):
                v.bn_stats(stats[:, g, :], xt[:, g * 384 : (g + 1) * 384])
            mv = spool.tile([P, 2], F32, tag="mv", name=f"mv{ii}")
            v.bn_aggr(mv[:], stats[:])
            sq = spool.tile([P, 1], F32, tag="sq", name=f"sq{ii}")
            act(sq[:], mv[:, 1:2], AF.Sqrt, bias=eps_t[:, 0:1])
            rstd = spool.tile([P, 1], F32, tag="rstd", name=f"rstd{ii}")
            v.reciprocal_approx_fast(rstd[:], sq[:])
            nx = nxpool.tile([P, D], BF16, tag="nx", name=f"nx{ii}")
            v.tensor_scalar(
                nx[:], xt[:], mv[:, 0:1], rstd[:, 0:1],
                op0=OP.subtract, op1=OP.mult,
            )
            nxs.append(nx)
        for ii in range(4):
            dmat(out=xn1T_bf[:, :, ii * P : (ii + 1) * P], in_=nxs[ii][:])
        for dc in range(NDC):
            if dc % 2 == 0:
                act(
                    xn1T8[b][:, dc, :], xn1T_bf[:, dc, :], AF.Identity,
                    bias=sh1_col[:, dc : dc + 1], scale=sp1_col[:, dc : dc + 1],
                )
            else:
                v.tensor_scalar(
                    xn1T8[b][:, dc, :], xn1T_bf[:, dc, :],
                    sp1_col[:, dc : dc + 1], sh1_col[:, dc : dc + 1],
                    op0=OP.mult, op1=OP.add,
                )
        # V projection for this block (4 k-chunks)
        for lc in range(4):
            ps_v = psB2.tile([P, D], F32, tag="psv", name=f"psv{lc}")
            for dc2 in range(NDC // 2):
                lhs = xn1T8[b][:, 2 * dc2 : 2 * dc2 + 2, lc * P : (lc + 1) * P]
                mm(ps_v[:, 0:512], lhs,
                   w8qkv[:, 2 * dc2 : 2 * dc2 + 2, 2 * D : 2 * D + 512],
                   start=(dc2 == 0), stop=(dc2 == NDC // 2 - 1), perf_mode=DR)
                mm(ps_v[:, 512:D], lhs,
                   w8qkv[:, 2 * dc2 : 2 * dc2 + 2, 2 * D + 512 : 3 * D],
                   start=(dc2 == 0), stop=(dc2 == NDC // 2 - 1), perf_mode=DR)
            kc = b * 4 + lc
            if lc % 2 == 0:
                act(v4r[:, kc, :, 0:HD],
                    ps_v.rearrange("p (h e) -> p h e", e=HD),
                    AF.Copy, scale=1.0 / SW_QKV)
            else:
                v.tensor_scalar(
                    v4r[:, kc, :, 0:HD],
                    ps_v.rearrange("p (h e) -> p h e", e=HD),
                    1.0 / SW_QKV, 0.0, op0=OP.mult, op1=OP.add,
                )
        if b == 0:
            # Q projection (local rows = block 0 thanks to the roll)
            for hp in range(NHP):
                ps_q = psB1.tile([P, LQ], F32, tag="psq", name=f"psq{hp}")
                for dc2 in range(NDC // 2):
                    mm(
                        ps_q[:],
                        w8qkv[:, 2 * dc2 : 2 * dc2 + 2, hp * P : (hp + 1) * P],
                        xn1T8[0][:, 2 * dc2 : 2 * dc2 + 2, :],
                        start=(dc2 == 0),
                        stop=(dc2 == NDC // 2 - 1),
                        perf_mode=DR,
                    )
                v.tensor_scalar(
                    qT_all[:, hp, :], ps_q[:], 1.0 / SW_QKV,
                    bq_col[:, hp : hp + 1], op0=OP.mult, op1=OP.add,
                )

    phB.close()  # release LN1 streaming pools before attention
    # ---------------- phase C: attention (+K emit, +adaln2) ---------------
    with ExitStack() as phC:
        pt_pool = phC.enter_context(tc.tile_pool(name="ptp", bufs=6))
        tiny = phC.enter_context(tc.tile_pool(name="tiny", bufs=2))
        psS = phC.enter_context(tc.tile_pool(name="psS", bufs=3, space="PSUM"))
        psO = phC.enter_context(tc.tile_pool(name="psO", bufs=1, space="PSUM"))

        def emit_K(hp):
            # K for head pair hp over the full sequence; bias dropped
            # (constant per query -> softmax invariant), 1/SW at convert.
            for b in range(NBL):
                ps_k = psS.tile([P, 1024], F32, tag="ps_s", name=f"psk{b}")
                for dc2 in range(NDC // 2):
                    mm(
                        ps_k[:, 0:LQ],
                        w8qkv[:, 2 * dc2 : 2 * dc2 + 2, D + hp * P : D + (hp + 1) * P],
                        xn1T8[b][:, 2 * dc2 : 2 * dc2 + 2, :],
                        start=(dc2 == 0),
                        stop=(dc2 == NDC // 2 - 1),
                        perf_mode=DR,
                    )
                if b % 2 == 0:
                    act(kT_all[:, hp, b * LQ : (b + 1) * LQ], ps_k[:, 0:LQ],
                        AF.Copy, scale=1.0 / SW_QKV)
                else:
                    v.tensor_scalar(
                        kT_all[:, hp, b * LQ : (b + 1) * LQ], ps_k[:, 0:LQ],
                        1.0 / SW_QKV, 0.0, op0=OP.mult, op1=OP.add,
                    )

        def emit_adaln2():
            # 8 tiles of 384 cols aligned to [sh2|sc2|g1|g2] groups
            with tc.tile_pool(name="wadp", bufs=1) as wadp:
                bg_row = wadp.tile([1, 4 * 384], BF16, name="bg_row")
                dma(out=bg_row[:], in_=t["bad2g"][:])
                sh_row = wadp.tile([1, 4 * 384], F32, name="sh_row")
                for half in range(4):
                    wad2 = wadp.tile(
                        [P, NDC, 2 * 384], BF16, tag="wad2", name=f"wad2_{half}"
                    )
                    dma2(
                        out=wad2[:],
                        in_=t["wad2"][:, :, half * 768 : (half + 1) * 768],
                    )
                    for t2 in range(2):
                        tt = half * 2 + t2
                        ps = psS.tile([P, 1024], F32, tag="ps_s", name=f"psa2_{tt}")
                        for dc in range(NDC):
                            mm(
                                ps[0:1, 0:384],
                                sc_bf[:, dc : dc + 1],
                                wad2[:, dc, t2 * 384 : (t2 + 1) * 384],
                                start=(dc == 0),
                                stop=(dc == NDC - 1),
                            )
                        if tt < 4:
                            # sh2|sc2: copy to an SBUF row (DMA can't read
                            # PSUM), then bounce through DRAM into columns
                            v.tensor_copy(
                                sh_row[:, tt * 384 : (tt + 1) * 384],
                                ps[0:1, 0:384],
                            )
                        else:
                            # g1|g2: copy to SBUF row, add bias, broadcast
                            g_sb = wadp.tile(
                                [1, 384], F32, tag="g_sb", name=f"g_sb{tt}"
                            )
                            v.tensor_copy(g_sb[:], ps[0:1, 0:384])
                            gi = tt - 4
                            v.tensor_add(
                                g_sb[:], g_sb[:],
                                bg_row[:, gi * 384 : (gi + 1) * 384],
                            )
                            dst = g1s_b if gi < 2 else g2s_b
                            col = (gi % 2) * 384
                            gp.partition_broadcast(
                                dst[:, col : col + 384], g_sb[:]
                            )
                dma(out=drB[:].rearrange("(o d) -> o d", o=1), in_=sh_row[:])
                sh2_raw = wadp.tile([P, NDC], F32, name="sh2_raw")
                dma(out=sh2_raw[:], in_=drB[0:D].rearrange("(c p) -> p c", p=P))
                sp2_raw = wadp.tile([P, NDC], F32, name="sp2_raw")
                dma(
                    out=sp2_raw[:],
                    in_=drB[D : 2 * D].rearrange("(c p) -> p c", p=P),
                )
                b2sh = wadp.tile([P, NDC], F32, name="b2sh")
                dma(out=b2sh[:], in_=t["bad2sh"][:, 0:NDC])
                b2sc = wadp.tile([P, NDC], F32, name="b2sc")
                dma(out=b2sc[:], in_=t["bad2sh"][:, NDC : 2 * NDC])
                v.tensor_add(sh2_col[:], sh2_raw[:], b2sh[:])
                v.tensor_add(sp2_col[:], sp2_raw[:], b2sc[:])

        emit_K(0)
        NK2 = NKC // 2
        for hp in range(NHP):
            ps_o = [
                psO.tile([VE, LQ], F32, tag=f"ps_o{dlt}", name=f"psO{hp}_{dlt}")
                for dlt in range(2)
            ]
            pending = None
            for kc2 in range(NK2):
                ps_s = [
                    psS.tile([P, 1024], F32, tag="ps_s", name=f"ps_s{_d}")
                    for _d in range(2)
                ]
                for j in range(2):
                    kc = 2 * kc2 + j
                    for dlt in range(2):
                        off = dlt * HD
                        mm(
                            ps_s[dlt][:, j * 512 : (j + 1) * 512],
                            kT_all[off : off + HD, hp, kc * P : (kc + 1) * P],
                            qT_all[off : off + HD, hp, :],
                            start=True,
                            stop=True,
                        )
                pts = []
                for dlt in range(2):
                    ptile = pt_pool.tile([P, 1024], FP8E5, tag="pt", name=f"pt{dlt}")
                    if dlt == 0 or kc2 % 8 == 0:
                        act(ptile[:], ps_s[dlt][:], AF.Exp, scale=0.125)
                    else:
                        v.tensor_scalar(
                            ptile.bitcast(I8)[:], ps_s[dlt][:], SCH_A, SCH_B,
                            op0=OP.mult, op1=OP.add,
                        )
                    pts.append(ptile)
                if pending is not None:
                    pk2, ppts = pending
                    for dlt in range(2):
                        h = 2 * hp + dlt
                        mm(
                            ps_o[dlt][:],
                            v4[:, 2 * pk2 : 2 * pk2 + 2, h * VE : (h + 1) * VE],
                            ppts[dlt].rearrange("p (j n) -> p j n", j=2)[:],
                            start=(pk2 == 0),
                            stop=False,
                            perf_mode=DR,
                        )
                pending = (kc2, pts)
                if kc2 == 7 and hp + 1 < NHP:
                    emit_K(hp + 1)
                if kc2 == 11 and hp == 0:
                    emit_adaln2()
            pk2, ppts = pending
            for dlt in range(2):
                h = 2 * hp + dlt
                mm(
                    ps_o[dlt][:],
                    v4[:, 2 * pk2 : 2 * pk2 + 2, h * VE : (h + 1) * VE],
                    ppts[dlt].rearrange("p (j n) -> p j n", j=2)[:],
                    start=False,
                    stop=True,
                    perf_mode=DR,
                )
            for dlt in range(2):
                off = dlt * HD
                zr = tiny.tile([1, LQ], F32, tag="zr", name=f"zr{dlt}")
                v.tensor_copy(zr[:], ps_o[dlt][HD : HD + 1, :])
                rz_f = tiny.tile([1, LQ], F32, tag="rz_f", name=f"rz_f{dlt}")
                v.reciprocal_approx_fast(rz_f[:], zr[:])
                rz_bf = tiny.tile([1, LQ], BF16, tag="rz_bf", name=f"rz_bf{dlt}")
                v.tensor_copy(rz_bf[:], rz_f[:])
                rzb = tiny.tile([P, LQ], BF16, tag="rzb", name=f"rzb{dlt}")
                gp.partition_broadcast(rzb[:], rz_bf[:])
                v.tensor_tensor(
                    catT8[off : off + HD, hp, :],
                    ps_o[dlt][0:HD, :],
                    rzb[0:HD, :],
                    op=OP.mult,
                )

    s_attn.close()  # free K/V/Q/xn1T8 space before the FFN weights land

    # -------- phase D: out-projection, residual, LN2 ---------------------
    with ExitStack() as phD:
        pool = phD.enter_context(tc.tile_pool(name="phD", bufs=2))
        spool = phD.enter_context(tc.tile_pool(name="spoolE", bufs=4))
        tpool2 = phD.enter_context(tc.tile_pool(name="tpool2", bufs=1))
        psD1 = phD.enter_context(tc.tile_pool(name="psD1", bufs=2, space="PSUM"))
        psD2 = phD.enter_context(tc.tile_pool(name="psD2", bufs=2, space="PSUM"))

        w8ao = pool.tile([P, NDC, D], FP8, name="w8ao")
        dma(out=w8ao[:], in_=t["w_ao8"].rearrange("(c p) m -> p c m", p=P))
        ba_sb = pool.tile([P, D], F32, name="ba_sb")
        dma(out=ba_sb[:], in_=t["b_attn_b"][:])
        bf2_sb = pool.tile([P, D], F32, name="bf2_sb")
        dma(out=bf2_sb[:], in_=t["b_ffn2_b"][:])
        v.tensor_tensor(xb_bias[:], ba_sb[:], g1s_b[:], op=OP.mult)
        v.tensor_tensor(x2b_bias[:], bf2_sb[:], g2s_b[:], op=OP.mult)
        xbl = [pool.tile([P, D], F32, name=f"xbl{q}") for q in range(NQC)]
        for q in range(NQC):
            v.tensor_add(xbl[q][:], x_loc[:, q, :], xb_bias[:])

        xn2T_bf = tpool2.tile([P, NDC, LQ], BF16)
        for qc in range(NQC):
            ps1 = psD1.tile([P, 512], F32)
            ps2 = psD2.tile([P, 256], F32)
            for cc2 in range(NDC // 2):
                lhs = catT8[:, 2 * cc2 : 2 * cc2 + 2, qc * P : (qc + 1) * P]
                mm(ps1[:], lhs, w8ao[:, 2 * cc2 : 2 * cc2 + 2, 0:512],
                   start=(cc2 == 0), stop=(cc2 == NDC // 2 - 1), perf_mode=DR)
                mm(ps2[:], lhs, w8ao[:, 2 * cc2 : 2 * cc2 + 2, 512:D],
                   start=(cc2 == 0), stop=(cc2 == NDC // 2 - 1), perf_mode=DR)
            gt = pool.tile([P, D], F32, tag="gt", name=f"gt{qc}")
            v.scalar_tensor_tensor(
                gt[:, 0:512], ps1[:], 1.0 / (SW_AO * SW_CAT), g1s_b[:, 0:512],
                op0=OP.mult, op1=OP.mult,
            )
            v.scalar_tensor_tensor(
                gt[:, 512:D], ps2[:], 1.0 / (SW_AO * SW_CAT), g1s_b[:, 512:D],
                op0=OP.mult, op1=OP.mult,
            )
            v.tensor_add(x2_loc[qc][:], gt[:], xbl[qc][:])
        for qc in range(NQC):
            stats = spool.tile([P, 2, 6], F32, tag="st2")
            for g in range(2):
                v.bn_stats(stats[:, g, :], x2_loc[qc][:, g * 384 : (g + 1) * 384])
            mv = spool.tile([P, 2], F32, tag="mv2", name=f"mv2_{qc}")
            v.bn_aggr(mv[:], stats[:])
            sq = spool.tile([P, 1], F32, tag="sq2")
            act(sq[:], mv[:, 1:2], AF.Sqrt, bias=eps_t[:, 0:1])
            rstd = spool.tile([P, 1], F32, tag="rstd2", name=f"rstd2_{qc}")
            v.reciprocal_approx_fast(rstd[:], sq[:])
            nx = spool.tile([P, D], BF16, tag="nx2", name=f"nx2_{qc}")
            v.tensor_scalar(
                nx[:], x2_loc[qc][:], mv[:, 0:1], rstd[:, 0:1],
                op0=OP.subtract, op1=OP.mult,
            )
            dmat(out=xn2T_bf[:, :, qc * P : (qc + 1) * P], in_=nx[:])
        for dc in range(NDC):
            if dc % 2 == 0:
                act(
                    xn2T8[:, dc, :], xn2T_bf[:, dc, :], AF.Identity,
                    bias=sh2_col[:, dc : dc + 1], scale=sp2_col[:, dc : dc + 1],
                )
            else:
                v.tensor_scalar(
                    xn2T8[:, dc, :], xn2T_bf[:, dc, :],
                    sp2_col[:, dc : dc + 1], sh2_col[:, dc : dc + 1],
                    op0=OP.mult, op1=OP.add,
                )

    # ---------------- phase F: FFN + gate + residual -> output ------------
    with ExitStack() as phF:
        wpool = phF.enter_context(tc.tile_pool(name="wffn", bufs=1))
        hpool = phF.enter_context(tc.tile_pool(name="hT", bufs=1))
        pool = phF.enter_context(tc.tile_pool(name="phF", bufs=2))
        psF1 = phF.enter_context(tc.tile_pool(name="psF1", bufs=3, space="PSUM"))
        psF2 = phF.enter_context(tc.tile_pool(name="psF2", bufs=2, space="PSUM"))

        w8f1 = wpool.tile([P, NDC, DM], FP8)
        wr = t["w_ffn18"].rearrange("(c p) m -> p c m", p=P)
        for q4 in range(4):
            dma2(
                out=w8f1[:, :, q4 * D : (q4 + 1) * D],
                in_=wr[:, :, q4 * D : (q4 + 1) * D],
            )
        bf1_col = wpool.tile([P, NMC], F32)
        dma(out=bf1_col[:], in_=t["b_ffn1_col"][:])
        w8f2 = wpool.tile([P, NMC, D], FP8)
        dma2(out=w8f2[:], in_=t["w_f28"].rearrange("(c p) m -> p c m", p=P))
        for q in range(NQC):
            v.tensor_add(x2_loc[q][:], x2_loc[q][:], x2b_bias[:])

        hT8 = hpool.tile([P, NMC, LQ], FP8)
        for mc in range(NMC):
            ps_h = psF1.tile([P, 512], F32, tag="mm512")
            for dc2 in range(NDC // 2):
                mm(
                    ps_h[:],
                    w8f1[:, 2 * dc2 : 2 * dc2 + 2, mc * P : (mc + 1) * P],
                    xn2T8[:, 2 * dc2 : 2 * dc2 + 2, :],
                    start=(dc2 == 0),
                    stop=(dc2 == NDC // 2 - 1),
                    perf_mode=DR,
                )
            act(
                hT8[:, mc, :], ps_h[:], AF.Gelu,
                bias=bf1_col[:, mc : mc + 1], scale=1.0 / SW_F1,
            )

        out_r = t["out"].rearrange("(n p) d -> n p d", p=P)
        for qc in range(NQC):
            ps1 = psF1.tile([P, 512], F32, tag="mm512")
            ps2 = psF2.tile([P, 256], F32)
            for mc2 in range(NMC // 2):
                lhs = hT8[:, 2 * mc2 : 2 * mc2 + 2, qc * P : (qc + 1) * P]
                mm(ps1[:], lhs, w8f2[:, 2 * mc2 : 2 * mc2 + 2, 0:512],
                   start=(mc2 == 0), stop=(mc2 == NMC // 2 - 1), perf_mode=DR)
                mm(ps2[:], lhs, w8f2[:, 2 * mc2 : 2 * mc2 + 2, 512:D],
                   start=(mc2 == 0), stop=(mc2 == NMC // 2 - 1), perf_mode=DR)
            gt = pool.tile([P, D], F32, tag="gt")
            v.scalar_tensor_tensor(
                gt[:, 0:512], ps1[:], 1.0 / SW_F2, g2s_b[:, 0:512],
                op0=OP.mult, op1=OP.mult,
            )
            v.scalar_tensor_tensor(
                gt[:, 512:D], ps2[:], 1.0 / SW_F2, g2s_b[:, 512:D],
                op0=OP.mult, op1=OP.mult,
            )
            ot = pool.tile([P, D], F32)
            v.tensor_add(ot[:], gt[:], x2_loc[qc][:])
            dma(out=out_r[qc], in_=ot[:])


def build_nc():
    nc = bacc.Bacc(None, target_bir_lowering=False, debug=False)
    t = _declare_params(nc)
    with tile.TileContext(nc) as tc:
        with ExitStack() as ctx:
            _build_body(nc, tc, ctx, t)
    nc.compile()
    return nc


_cache = {}


def _prep_in_maps(inputs):
    E4 = ml_dtypes.float8_e4m3fn
    f32 = lambda a: np.ascontiguousarray(np.asarray(a, np.float32))
    q8 = lambda a, s: np.ascontiguousarray(
        (np.asarray(a, np.float32) * s).astype(E4)
    )
    x = f32(inputs["x"]).reshape(L, D)
    cond = f32(inputs["cond"]).reshape(D)
    b_qkv = f32(inputs["b_qkv"]).reshape(3 * D)
    w_ao = f32(inputs["w_attn_out"])
    b_attn_eff = f32(inputs["b_attn_out"]).reshape(D) + b_qkv[2 * D :] @ w_ao
    w_ad1 = f32(inputs["w_adaln1"])  # [D, 3D]: sh1|sc1|g1
    w_ad2 = f32(inputs["w_adaln2"])
    b_ad1 = f32(inputs["b_adaln1"]).reshape(3 * D)
    b_ad2 = f32(inputs["b_adaln2"]).reshape(3 * D)
    wad1 = w_ad1[:, 0 : 2 * D]
    wad2 = np.concatenate(
        [w_ad2[:, 0 : 2 * D], w_ad1[:, 2 * D :], w_ad2[:, 2 * D :]], axis=1
    )
    bad1 = b_ad1[0 : 2 * D]
    b2shc = np.zeros((P, 2 * NDC), np.float32)
    b2shc[:, 0:NDC] = b_ad2[0:D].reshape(NDC, P).T
    b2shc[:, NDC : 2 * NDC] = b_ad2[D : 2 * D].reshape(NDC, P).T + 1.0
    bad2g = np.concatenate([b_ad1[2 * D :], b_ad2[2 * D :]])
    common = {
        "cond_t": np.ascontiguousarray(cond.reshape(NDC, P).T),
        "wad1": np.ascontiguousarray(
            wad1.reshape(NDC, P, 2 * D).transpose(1, 0, 2)
        ).astype(ml_dtypes.bfloat16),
        "wad2": np.ascontiguousarray(
            wad2.reshape(NDC, P, 4 * D).transpose(1, 0, 2)
        ).astype(ml_dtypes.bfloat16),
        "bad1": np.ascontiguousarray(bad1[None]),
        "bad2sh": np.ascontiguousarray(b2shc),
        "bad2g": np.ascontiguousarray(bad2g[None]).astype(ml_dtypes.bfloat16),
        "w_qkv8": q8(inputs["w_qkv"], SW_QKV),
        "b_q_col": np.ascontiguousarray(b_qkv[:D].reshape(NDC, P).T),
        "w_ao8": q8(w_ao, SW_AO),
        "b_attn_b": np.ascontiguousarray(np.broadcast_to(b_attn_eff, (P, D))),
        "w_ffn18": q8(inputs["w_ffn1"], SW_F1),
        "b_ffn1_col": np.ascontiguousarray(
            f32(inputs["b_ffn1"]).reshape(NMC, P).T
        ),
        "w_f28": q8(inputs["w_ffn2"], SW_F2),
        "b_ffn2_b": np.ascontiguousarray(
            np.broadcast_to(f32(inputs["b_ffn2"]).reshape(D), (P, D))
        ),
    }
    in_maps = []
    for c in range(NCORES):
        m = dict(common)
        xr = np.roll(x, -c * LQ, axis=0)
        m["x"] = np.ascontiguousarray(xr[:LQ])
        m["x_bf"] = np.ascontiguousarray(xr.astype(ml_dtypes.bfloat16))
        in_maps.append(m)
    return in_maps


def kernel(**inputs):
    if "nc" not in _cache:
        _cache["nc"] = build_nc()
    nc = _cache["nc"]
    in_maps = _prep_in_maps(inputs)
    res = run_bass_kernel_spmd(nc, in_maps, list(range(NCORES)))
    out = np.concatenate([res.results[c]["out"] for c in range(NCORES)], axis=0)
    return out.reshape(1, L, D).astype(np.float32)


if __name__ == "__main__":
    rng = np.random.default_rng(0)
    fake = {
        "x": rng.standard_normal((1, L, D), dtype=np.float32),
        "cond": rng.standard_normal((1, D), dtype=np.float32),
        "w_adaln1": rng.standard_normal((D, 3 * D), dtype=np.float32) * 0.02,
        "b_adaln1": np.zeros(3 * D, np.float32),
        "w_qkv": rng.standard_normal((D, 3 * D), dtype=np.float32) * D**-0.5,
        "b_qkv": np.zeros(3 * D, np.float32),
        "w_attn_out": rng.standard_normal((D, D), dtype=np.float32) * D**-0.5,
        "b_attn_out": np.zeros(D, np.float32),
        "w_adaln2": rng.standard_normal((D, 3 * D), dtype=np.float32) * 0.02,
        "b_adaln2": np.zeros(3 * D, np.float32),
        "w_ffn1": rng.standard_normal((D, DM), dtype=np.float32) * D**-0.5,
        "b_ffn1": np.zeros(DM, np.float32),
        "w_ffn2": rng.standard_normal((DM, D), dtype=np.float32) * DM**-0.5,
        "b_ffn2": np.zeros(D, np.float32),
    }
    out = kernel(**fake)
    print("out", out.shape, out.dtype, np.abs(out).max())
